# revision 29
# baseline (speedup 1.0000x reference)
"""Bass/Trainium2 kernel for MultiHeadAttentionWithDSA (sparse attention with
lightning-indexer top-64 key selection), sharded over 8 NeuronCores.

Sharding: core = b*4 + g  (b in {0,1} batch, g in {0..3} head-group of 4 heads).
Each core computes a partial output  ctx_g @ Wo[g*256:(g+1)*256, :]  for its
batch; the host sums the 4 partials per batch and adds the bias.

All matmuls run in true fp32 (4 cycles/row) because the top-64 selection must
match the fp32 reference's ordering exactly at the boundaries.
"""

import numpy as np

import concourse.bacc as bacc
import concourse.bass as bass
import concourse.mybir as mybir
import concourse.tile as tile
from concourse import masks
from concourse.bass_utils import run_bass_kernel_spmd

F32 = mybir.dt.float32
F32R = mybir.dt.float32r
USE_F32R = True
MMDT = F32R if USE_F32R else F32
AF = mybir.ActivationFunctionType
ALU = mybir.AluOpType

B, T, D = 2, 1024, 1024
H, HD = 16, 64          # total heads, head dim
HG = 4                  # heads per core
HI, IHD = 4, 64         # index heads, index head dim
TOPK = 64
NCHUNK = T // 128       # 8 token chunks of 128
NEG = -3.0e30           # causal-invalid marker (additive mask value)
SENT = -1.0e30          # match_replace sentinel (distinct from NEG)

_NEFF_CACHE = "/var/tmp/bass-neff-cache"


def _install_neff_cache():
    """walrus compile output cache keyed on BIR hash (compiles are minutes)."""
    import hashlib
    import os
    import shutil

    import concourse.bass2jax as b2j

    if getattr(b2j, "_dsa_neff_cache_installed", False):
        return
    orig = b2j.compile_bir_kernel

    def cached(bir_json, tmpdir, neff_name="file.neff"):
        try:
            h = hashlib.sha256(
                bir_json if isinstance(bir_json, bytes) else bir_json.encode()
            ).hexdigest()[:24]
            os.makedirs(_NEFF_CACHE, exist_ok=True)
            hit = os.path.join(_NEFF_CACHE, h + ".neff")
            if os.path.exists(hit):
                dst = os.path.join(tmpdir, neff_name)
                shutil.copyfile(hit, dst)
                return dst
            neff = orig(bir_json, tmpdir, neff_name)
            shutil.copyfile(neff, hit + ".tmp")
            os.replace(hit + ".tmp", hit)
            return neff
        except OSError:
            return orig(bir_json, tmpdir, neff_name)

    b2j.compile_bir_kernel = cached
    b2j._dsa_neff_cache_installed = True


def build_kernel(tc, out_ap, x_ap, wq_ap, wk_ap, wv_ap, wo_ap, wi_ap):
    """Emit the per-core kernel. All APs are DRAM tensors:
    x [1024,1024], wq/wk/wv [1024,256], wo [256,1024],
    wi [1024,324] = concat(Wqi[1024,256], Wki[1024,64], Ww[1024,4]).
    out [1024,1024] partial (pre-bias, pre-reduction over head groups).
    """
    nc = tc.nc
    from contextlib import ExitStack
    stack = ExitStack()

    const_pool = stack.enter_context(tc.tile_pool(name="const", bufs=1))
    ident = const_pool.tile([128, 128], F32)
    masks.make_identity(nc, ident[:])
    causal = const_pool.tile([128, 128], F32)
    masks.make_causal_mask(nc, causal[:], mask_val=NEG)

    w_pool = stack.enter_context(tc.tile_pool(name="weights", bufs=1))
    wq_sb = w_pool.tile([128, 8 * 256], MMDT)
    wk_sb = w_pool.tile([128, 8 * 256], MMDT)
    wv_sb = w_pool.tile([128, 8 * 256], MMDT)
    wi_sb = w_pool.tile([128, 8 * 324], F32)
    wo_sb = w_pool.tile([128, 2 * 1024], MMDT)
    with tc.tile_pool(name="wload", bufs=2) as wload:
        for j in range(8):
            nc.sync.dma_start(wi_sb[:, j * 324:(j + 1) * 324], wi_ap[j * 128:(j + 1) * 128, :])
        for (ap_, dst_) in ((wq_ap, wq_sb), (wk_ap, wk_sb), (wv_ap, wv_sb)):
            for j in range(8):
                wt = wload.tile([128, 256], F32, name="wt", tag="wt")
                nc.sync.dma_start(wt[:], ap_[j * 128:(j + 1) * 128, :])
                nc.scalar.copy(dst_[:, j * 256:(j + 1) * 256], wt[:])
        for ck in range(2):
            wt2 = wload.tile([128, 1024], F32, name="wt2", tag="wt2")
            nc.sync.dma_start(wt2[:], wo_ap[ck * 128:(ck + 1) * 128, :])
            nc.scalar.copy(wo_sb[:, ck * 1024:(ck + 1) * 1024], wt2[:])

    act_pool = stack.enter_context(tc.tile_pool(name="acts", bufs=1))
    qT = act_pool.tile([128, 2 * 1024], MMDT)    # heads (2m,2m+1) rows, tokens free
    kT = act_pool.tile([128, 2 * 1024], MMDT)
    qiT = act_pool.tile([128, 2 * 1024], F32)
    kiw = act_pool.tile([128, 1024], F32)        # rows 0-63 kiT, 64-67 wT logits
    kiw2 = act_pool.tile([128, 1024], F32)       # rows 64-127: copy of kiT (odd index heads)
    v_sb = act_pool.tile([128, 8 * 256], MMDT)    # [s-chunk sc] at cols sc*256, head cols inside
    w8 = act_pool.tile([128, 32], F32)           # softmax(x@Ww)/8, chunk i at cols 4i
    mask_tiles = [act_pool.tile([128, (i + 1) * 128], F32, name=f"mask{i}", tag=f"mask{i}") for i in range(NCHUNK)]

    # ---- Phase A: load x, build xT via PE transposes ----
    with tc.tile_pool(name="tp_ps", bufs=2, space="PSUM") as tp_ps, \
         tc.tile_pool(name="mm_ps", bufs=4, space="PSUM") as mm_ps:
      with tc.tile_pool(name="xscope", bufs=1) as xscope, \
           tc.tile_pool(name="xtok", bufs=2) as xtok_pool:
        xT = xscope.tile([128, 8 * 1024], F32)   # [d-chunk j] at cols j*1024, feature-major
        xTr = xscope.tile([128, 8 * 1024], MMDT)  # rounded shadow for fp32r matmuls

        for i in range(NCHUNK):
            xt = xtok_pool.tile([128, 1024], F32, tag="xtok")
            nc.sync.dma_start(xt[:], x_ap[i * 128:(i + 1) * 128, :])
            for j in range(8):
                pt = tp_ps.tile([128, 128], F32, tag="tp")
                nc.tensor.transpose(pt[:], xt[:, j * 128:(j + 1) * 128], ident[:])
                nc.scalar.copy(xT[:, j * 1024 + i * 128: j * 1024 + (i + 1) * 128], pt[:])
        for j in range(8):
            nc.scalar.copy(xTr[:, j * 1024:(j + 1) * 1024], xT[:, j * 1024:(j + 1) * 1024])

        # ---- Phase B: projections (contract d over 8 chunks) ----
        # qT/kT/qiT: out [128 (2 heads x 64), t512] ; lhsT = W[:, m*128:+128]
        for (wsb, dst) in ((wq_sb, qT), (wk_sb, kT)):
            for m in range(2):
                for tg in range(2):
                    ps = mm_ps.tile([128, 512], F32, tag="mm")
                    for j in range(8):
                        nc.tensor.matmul(
                            ps[:],
                            wsb[:, j * 256 + m * 128: j * 256 + (m + 1) * 128],
                            xTr[:, j * 1024 + tg * 512: j * 1024 + (tg + 1) * 512],
                            start=(j == 0), stop=(j == 7))
                    nc.scalar.copy(dst[:, m * 1024 + tg * 512: m * 1024 + (tg + 1) * 512], ps[:])
        for m in range(2):  # qiT
            for tg in range(2):
                ps = mm_ps.tile([128, 512], F32, tag="mm")
                for j in range(8):
                    nc.tensor.matmul(
                        ps[:],
                        wi_sb[:, j * 324 + m * 128: j * 324 + (m + 1) * 128],
                        xT[:, j * 1024 + tg * 512: j * 1024 + (tg + 1) * 512],
                        start=(j == 0), stop=(j == 7))
                nc.scalar.copy(qiT[:, m * 1024 + tg * 512: m * 1024 + (tg + 1) * 512], ps[:])
        for tg in range(2):  # kiT + wT logits (68 cols of wi)
            ps = mm_ps.tile([128, 512], F32, tag="mm")
            for j in range(8):
                nc.tensor.matmul(
                    ps[0:68, :],
                    wi_sb[:, j * 324 + 256: j * 324 + 324],
                    xT[:, j * 1024 + tg * 512: j * 1024 + (tg + 1) * 512],
                    start=(j == 0), stop=(j == 7))
            nc.scalar.copy(kiw[0:68, tg * 512:(tg + 1) * 512], ps[0:68, :])
        nc.sync.dma_start(kiw2[64:128, :], kiw[0:64, :])
        # v natural layout: out [s128, 256]
        for sc in range(NCHUNK):
            ps = mm_ps.tile([128, 512], F32, tag="mm")
            for j in range(8):
                nc.tensor.matmul(
                    ps[:, 0:256],
                    xTr[:, j * 1024 + sc * 128: j * 1024 + (sc + 1) * 128],
                    wv_sb[:, j * 256:(j + 1) * 256],
                    start=(j == 0), stop=(j == 7))
            nc.scalar.copy(v_sb[:, sc * 256:(sc + 1) * 256], ps[:, 0:256])

        # w softmax per chunk: transpose wT logits [4, t128] -> [t128, 4]
        for i in range(NCHUNK):
            pw = tp_ps.tile([128, 128], F32, tag="tp")
            nc.tensor.transpose(pw[:, 0:4], kiw[64:68, i * 128:(i + 1) * 128], ident[64:68, 64:68])
            wexp = act_pool.tile([128, 4], F32, tag="wexp", bufs=2)
            wden = act_pool.tile([128, 1], F32, tag="wden", bufs=2)
            nc.scalar.activation(wexp[:], pw[:, 0:4], AF.Exp, accum_out=wden[:])
            wrec = act_pool.tile([128, 1], F32, tag="wrec", bufs=2)
            nc.vector.reciprocal(wrec[:], wden[:])
            nc.vector.tensor_scalar(w8[:, i * 4:(i + 1) * 4], wexp[:], wrec[:], 0.125,
                                    op0=ALU.mult, op1=ALU.mult)

      # ---- Phases C+D, interleaved per t-group: topk(tg+1) overlaps attention(tg) ----
      with tc.tile_pool(name="idx", bufs=2) as idx_pool, \
           tc.tile_pool(name="attn", bufs=1) as attn_pool, \
           tc.tile_pool(name="attn2", bufs=2) as attn2_pool, \
           tc.tile_pool(name="ctx_ps", bufs=2, space="PSUM") as ctx_ps:
            ctxT = attn_pool.tile([128, 2 * 1024], MMDT)  # [ck] at cols ck*1024

            def emit_idx(i):
                n_s = (i + 1) * 128
                work = idx_pool.tile([128, 1024], F32, name="work", tag="work")
                for h in range(HI):
                    m, r = h // 2, (h % 2) * 64
                    dst = work if h == 0 else idx_pool.tile([128, 1024], F32, name="aw", tag="aw")
                    for grp in range((n_s + 511) // 512):
                        ns0, ns1 = grp * 512, min(n_s, (grp + 1) * 512)
                        ps = mm_ps.tile([128, 512], F32, name="ps", tag="mm")
                        ki_rhs = kiw[0:64, ns0:ns1] if r == 0 else kiw2[64:128, ns0:ns1]
                        nc.tensor.matmul(
                            ps[:, 0:ns1 - ns0],
                            qiT[r:r + 64, m * 1024 + i * 128: m * 1024 + (i + 1) * 128],
                            ki_rhs,
                            start=True, stop=True)
                        nc.scalar.activation(dst[:, ns0:ns1], ps[:, 0:ns1 - ns0], AF.Relu,
                                             scale=w8[:, i * 4 + h: i * 4 + h + 1])
                    if h > 0:
                        nc.gpsimd.tensor_tensor(work[:, 0:n_s], work[:, 0:n_s], dst[:, 0:n_s], op=ALU.add)
                nc.gpsimd.tensor_tensor(work[:, i * 128:(i + 1) * 128],
                                        work[:, i * 128:(i + 1) * 128], causal[:], op=ALU.add)
                tmax = idx_pool.tile([128, 8], F32, name="tmax", tag="tmax")
                for _ in range(8):
                    nc.vector.max(tmax[:], work[:, 0:n_s])
                    nc.vector.match_replace(work[:, 0:n_s], tmax[:], work[:, 0:n_s], SENT)
                mk = mask_tiles[i]
                nc.vector.tensor_scalar(mk[:], work[:, 0:n_s], SENT, NEG,
                                        op0=ALU.not_equal, op1=ALU.mult)
                nc.gpsimd.tensor_tensor(mk[:, i * 128:(i + 1) * 128],
                                        mk[:, i * 128:(i + 1) * 128], causal[:], op=ALU.add)

            def emit_attn_tg(tg):
                i_lo, i_hi = tg * 4, tg * 4 + 4
                for h in range(HG):
                    m, r = h // 2, (h % 2) * 64
                    probT = [attn_pool.tile([128, 512], MMDT, name=f"probT{sc}", tag=f"probT{sc}", bufs=2) for sc in range(i_hi)]
                    for i in range(i_lo, i_hi):
                        n_s = (i + 1) * 128
                        sc_sb = attn2_pool.tile([128, 1024], F32, name="sc_sb", tag="sc")
                        for grp in range((n_s + 511) // 512):
                            ns0, ns1 = grp * 512, min(n_s, (grp + 1) * 512)
                            ps = mm_ps.tile([128, 512], F32, name="ps", tag="mm")
                            nc.tensor.matmul(
                                ps[:, 0:ns1 - ns0],
                                qT[r:r + 64, m * 1024 + i * 128: m * 1024 + (i + 1) * 128],
                                kT[r:r + 64, m * 1024 + ns0: m * 1024 + ns1],
                                start=True, stop=True)
                            nc.vector.tensor_tensor(sc_sb[:, ns0:ns1], ps[:, 0:ns1 - ns0],
                                                    mask_tiles[i][:, ns0:ns1], op=ALU.add)
                        scr = attn2_pool.tile([128, 1024], F32, name="scr", tag="scr")
                        den = attn2_pool.tile([128, 1], F32, name="den", tag="den")
                        nc.scalar.activation(scr[:, 0:n_s], sc_sb[:, 0:n_s], AF.Exp,
                                             scale=0.125, accum_out=den[:])
                        rec = attn2_pool.tile([128, 1], F32, name="rec", tag="rec")
                        nc.vector.reciprocal(rec[:], den[:])
                        nc.gpsimd.tensor_scalar(sc_sb[:, 0:n_s], scr[:, 0:n_s], rec[:], None,
                                                op0=ALU.mult)
                        for sc in range(i + 1):
                            pt = tp_ps.tile([128, 128], F32, name="pt", tag="tp")
                            nc.tensor.transpose(pt[:], sc_sb[:, sc * 128:(sc + 1) * 128], ident[:])
                            nc.scalar.copy(probT[sc][:, (i - i_lo) * 128:(i - i_lo + 1) * 128], pt[:])
                    pc = ctx_ps.tile([64, 512], F32, name="pc", tag="ctx")
                    n_sc = i_hi
                    for sc in range(n_sc):
                        off = max(sc - i_lo, 0) * 128
                        nc.tensor.matmul(
                            pc[:, off:512],
                            v_sb[:, sc * 256 + h * 64: sc * 256 + (h + 1) * 64],
                            probT[sc][:, off:512],
                            start=(sc == 0), stop=(sc == n_sc - 1))
                    ck, rr = h // 2, (h % 2) * 64
                    nc.scalar.copy(ctxT[rr:rr + 64, ck * 1024 + tg * 512: ck * 1024 + (tg + 1) * 512], pc[:])
                for i in range(i_lo, i_hi):
                    out_sb = attn2_pool.tile([128, 1024], F32, name="out_sb", tag="out")
                    for og in range(2):
                        ps = mm_ps.tile([128, 512], F32, name="ps", tag="mm")
                        for ck in range(2):
                            nc.tensor.matmul(
                                ps[:],
                                ctxT[:, ck * 1024 + i * 128: ck * 1024 + (i + 1) * 128],
                                wo_sb[:, ck * 1024 + og * 512: ck * 1024 + (og + 1) * 512],
                                start=(ck == 0), stop=(ck == 1))
                        nc.scalar.copy(out_sb[:, og * 512:(og + 1) * 512], ps[:])
                    nc.sync.dma_start(out_ap[i * 128:(i + 1) * 128, :], out_sb[:])

            for tg in range(2):
                for i in range(tg * 4, tg * 4 + 4):
                    emit_idx(i)
                emit_attn_tg(tg)

    stack.close()


def _build_nc(loop=0):
    nc = bacc.Bacc("TRN2")
    x = nc.dram_tensor("x", [T, D], F32, kind="ExternalInput")
    wq = nc.dram_tensor("wq", [D, 256], F32, kind="ExternalInput")
    wk = nc.dram_tensor("wk", [D, 256], F32, kind="ExternalInput")
    wv = nc.dram_tensor("wv", [D, 256], F32, kind="ExternalInput")
    wo = nc.dram_tensor("wo", [256, D], F32, kind="ExternalInput")
    wi = nc.dram_tensor("wi", [D, 324], F32, kind="ExternalInput")
    out = nc.dram_tensor("out", [T, D], F32, kind="ExternalOutput")
    with tile.TileContext(nc) as tc:
        if loop:
            with tc.For_i(0, loop, 1):
                build_kernel(tc, out.ap(), x.ap(), wq.ap(), wk.ap(), wv.ap(), wo.ap(), wi.ap())
        else:
            build_kernel(tc, out.ap(), x.ap(), wq.ap(), wk.ap(), wv.ap(), wo.ap(), wi.ap())
    nc.compile()
    return nc


def kernel(x, Wq, Wk, Wv, Wo, bo, Wqi, Wki, Ww, _trace=False):
    _install_neff_cache()
    x, Wq, Wk, Wv, Wo, bo, Wqi, Wki, Ww = (
        np.ascontiguousarray(np.asarray(a, np.float32))
        for a in (x, Wq, Wk, Wv, Wo, bo, Wqi, Wki, Ww))
    wi = np.ascontiguousarray(np.concatenate([Wqi, Wki, Ww], axis=1))
    nc = _build_nc()
    in_maps = []
    for b in range(B):
        for g in range(4):
            c = slice(g * 256, (g + 1) * 256)
            in_maps.append({
                "x": np.ascontiguousarray(x[b]),
                "wq": np.ascontiguousarray(Wq[:, c]),
                "wk": np.ascontiguousarray(Wk[:, c]),
                "wv": np.ascontiguousarray(Wv[:, c]),
                "wo": np.ascontiguousarray(Wo[c, :]),
                "wi": wi,
            })
    res = run_bass_kernel_spmd(nc, in_maps, core_ids=list(range(8)), trace=_trace)
    outs = [r["out"] for r in res.results]
    full = np.stack([sum(outs[b * 4:(b + 1) * 4]) + bo for b in range(B)], axis=0)
    full = full.astype(np.float32)
    if _trace:
        return full, res
    return full


def _make_in_maps(x, Wq, Wk, Wv, Wo, Wqi, Wki, Ww):
    wi = np.ascontiguousarray(np.concatenate([Wqi, Wki, Ww], axis=1))
    in_maps = []
    for b in range(B):
        for g in range(4):
            c = slice(g * 256, (g + 1) * 256)
            in_maps.append({
                "x": np.ascontiguousarray(x[b]),
                "wq": np.ascontiguousarray(Wq[:, c]),
                "wk": np.ascontiguousarray(Wk[:, c]),
                "wv": np.ascontiguousarray(Wv[:, c]),
                "wo": np.ascontiguousarray(Wo[c, :]),
                "wi": wi,
            })
    return in_maps


def bench_exec_ns(inputs, iters=10, loop=256):
    """Per-iteration device time: the kernel body loops `loop` times inside one
    NEFF; dispatch-overhead floor (loop=1 variant) is subtracted via the slope
    between two loop counts. Returns ns per kernel iteration."""
    lo = max(1, loop // 8)
    t_hi = _bench_exec_wall(inputs, iters, loop)
    t_lo = _bench_exec_wall(inputs, iters, lo)
    return (t_hi - t_lo) / (loop - lo) * 1e9


def _bench_exec_wall(inputs, iters, loop):
    import time

    import jax
    from jax.experimental.shard_map import shard_map
    from jax.sharding import Mesh, NamedSharding, PartitionSpec

    import concourse.bass2jax as b2j

    _install_neff_cache()
    b2j.install_neuronx_cc_hook()
    nc = _build_nc(loop=loop)
    ins = {k: np.ascontiguousarray(np.asarray(v, np.float32)) for k, v in inputs.items()}
    in_maps = _make_in_maps(ins["x"], ins["Wq"], ins["Wk"], ins["Wv"], ins["Wo"],
                            ins["Wqi"], ins["Wki"], ins["Ww"])

    partition_name = nc.partition_id_tensor.name if nc.partition_id_tensor else None
    in_names, out_names, out_avals, zero_outs = [], [], [], []
    for alloc in nc.m.functions[0].allocations:
        if not isinstance(alloc, mybir.MemoryLocationSet):
            continue
        name = alloc.memorylocations[0].name
        if alloc.kind == "ExternalInput":
            if name != partition_name:
                in_names.append(name)
        elif alloc.kind == "ExternalOutput":
            shape = tuple(alloc.tensor_shape)
            dtype = mybir.dt.np(alloc.dtype)
            out_names.append(name)
            out_avals.append(jax.core.ShapedArray(shape, dtype))
            zero_outs.append(np.zeros(shape, dtype))
    n_params = len(in_names)
    all_in_names = list(in_names) + list(out_names)
    if partition_name is not None:
        all_in_names.append(partition_name)

    def _body(*args):
        operands = list(args)
        if partition_name is not None:
            operands.append(b2j.partition_id_tensor())
        outs = b2j._bass_exec_p.bind(
            *operands,
            out_avals=tuple(out_avals),
            in_names=tuple(all_in_names),
            out_names=tuple(out_names),
            lowering_input_output_aliases=(),
            sim_require_finite=True,
            sim_require_nnan=True,
            nc=nc,
        )
        return tuple(outs)

    n_cores = len(in_maps)
    devices = jax.devices()[:n_cores]
    mesh = Mesh(np.asarray(devices), ("core",))
    in_specs = (PartitionSpec("core"),) * (n_params + len(out_names))
    out_specs = (PartitionSpec("core"),) * len(out_names)
    fn = jax.jit(shard_map(_body, mesh=mesh, in_specs=in_specs,
                           out_specs=out_specs, check_rep=False))
    sharding = NamedSharding(mesh, PartitionSpec("core"))
    dev_args = [
        jax.device_put(
            np.concatenate([np.asarray(in_maps[c][nm]) for c in range(n_cores)], axis=0),
            sharding)
        for nm in in_names
    ] + [
        jax.device_put(np.concatenate([z] * n_cores, axis=0), sharding)
        for z in zero_outs
    ]
    r = fn(*dev_args)
    jax.block_until_ready(r)
    times = []
    for _ in range(iters):
        t0 = time.perf_counter()
        r = fn(*dev_args)
        jax.block_until_ready(r)
        times.append(time.perf_counter() - t0)
    return min(times)


if __name__ == "__main__":
    rng = np.random.default_rng(0)
    ins = {
        "x": rng.standard_normal((B, T, D)).astype(np.float32),
        "Wq": (rng.standard_normal((D, D)) * 0.02).astype(np.float32),
        "Wk": (rng.standard_normal((D, D)) * 0.02).astype(np.float32),
        "Wv": (rng.standard_normal((D, D)) * 0.02).astype(np.float32),
        "Wo": (rng.standard_normal((D, D)) * 0.02).astype(np.float32),
        "bo": np.zeros(D, np.float32),
        "Wqi": (rng.standard_normal((D, HI * IHD)) * 0.02).astype(np.float32),
        "Wki": (rng.standard_normal((D, IHD)) * 0.02).astype(np.float32),
        "Ww": (rng.standard_normal((D, HI)) * 0.02).astype(np.float32),
    }
    out = kernel(**ins)
    print("out", out.shape, out.dtype, float(np.abs(out).max()))


# revision 30
# speedup vs baseline: 1.0721x; 1.0721x over previous
"""Bass/Trainium2 kernel for MultiHeadAttentionWithDSA (sparse attention with
lightning-indexer top-64 key selection), sharded over 8 NeuronCores.

Sharding: core = b*4 + g  (b in {0,1} batch, g in {0..3} head-group of 4 heads).
Each core computes a partial output  ctx_g @ Wo[g*256:(g+1)*256, :]  for its
batch; the host sums the 4 partials per batch and adds the bias.

All matmuls run in true fp32 (4 cycles/row) because the top-64 selection must
match the fp32 reference's ordering exactly at the boundaries.
"""

import numpy as np

import concourse.bacc as bacc
import concourse.bass as bass
import concourse.mybir as mybir
import concourse.tile as tile
from concourse import masks
from concourse.bass_utils import run_bass_kernel_spmd

F32 = mybir.dt.float32
F32R = mybir.dt.float32r
USE_F32R = True
MMDT = F32R if USE_F32R else F32
AF = mybir.ActivationFunctionType
ALU = mybir.AluOpType

B, T, D = 2, 1024, 1024
H, HD = 16, 64          # total heads, head dim
HG = 4                  # heads per core
HI, IHD = 4, 64         # index heads, index head dim
TOPK = 64
NCHUNK = T // 128       # 8 token chunks of 128
NEG = -3.0e30           # causal-invalid marker (additive mask value)
SENT = -1.0e30          # match_replace sentinel (distinct from NEG)

_NEFF_CACHE = "/var/tmp/bass-neff-cache"


def _install_neff_cache():
    """walrus compile output cache keyed on BIR hash (compiles are minutes)."""
    import hashlib
    import os
    import shutil

    import concourse.bass2jax as b2j

    if getattr(b2j, "_dsa_neff_cache_installed", False):
        return
    orig = b2j.compile_bir_kernel

    def cached(bir_json, tmpdir, neff_name="file.neff"):
        try:
            h = hashlib.sha256(
                bir_json if isinstance(bir_json, bytes) else bir_json.encode()
            ).hexdigest()[:24]
            os.makedirs(_NEFF_CACHE, exist_ok=True)
            hit = os.path.join(_NEFF_CACHE, h + ".neff")
            if os.path.exists(hit):
                dst = os.path.join(tmpdir, neff_name)
                shutil.copyfile(hit, dst)
                return dst
            neff = orig(bir_json, tmpdir, neff_name)
            shutil.copyfile(neff, hit + ".tmp")
            os.replace(hit + ".tmp", hit)
            return neff
        except OSError:
            return orig(bir_json, tmpdir, neff_name)

    b2j.compile_bir_kernel = cached
    b2j._dsa_neff_cache_installed = True


def build_kernel(tc, out_ap, x_ap, wq_ap, wk_ap, wv_ap, wo_ap, wi_ap):
    """Emit the per-core kernel. All APs are DRAM tensors:
    x [1024,1024], wq/wk/wv [1024,256], wo [256,1024],
    wi [1024,324] = concat(Wqi[1024,256], Wki[1024,64], Ww[1024,4]).
    out [1024,1024] partial (pre-bias, pre-reduction over head groups).
    """
    nc = tc.nc
    from contextlib import ExitStack
    stack = ExitStack()

    const_pool = stack.enter_context(tc.tile_pool(name="const", bufs=1))
    ident = const_pool.tile([128, 128], F32)
    masks.make_identity(nc, ident[:])
    causal = const_pool.tile([128, 128], F32)
    masks.make_causal_mask(nc, causal[:], mask_val=NEG)

    w_pool = stack.enter_context(tc.tile_pool(name="weights", bufs=1))
    wq_sb = w_pool.tile([128, 8 * 256], MMDT)
    wk_sb = w_pool.tile([128, 8 * 256], MMDT)
    wv_sb = w_pool.tile([128, 8 * 256], MMDT)
    wo_sb = w_pool.tile([128, 2 * 1024], MMDT)
    with tc.tile_pool(name="wload", bufs=2) as wload:
        for (ap_, dst_) in ((wq_ap, wq_sb), (wk_ap, wk_sb), (wv_ap, wv_sb)):
            for j in range(8):
                wt = wload.tile([128, 256], F32, name="wt", tag="wt")
                nc.sync.dma_start(wt[:], ap_[j * 128:(j + 1) * 128, :])
                nc.scalar.copy(dst_[:, j * 256:(j + 1) * 256], wt[:])
        for ck in range(2):
            wt2 = wload.tile([128, 1024], F32, name="wt2", tag="wt2")
            nc.sync.dma_start(wt2[:], wo_ap[ck * 128:(ck + 1) * 128, :])
            nc.scalar.copy(wo_sb[:, ck * 1024:(ck + 1) * 1024], wt2[:])

    act_pool = stack.enter_context(tc.tile_pool(name="acts", bufs=1))
    qT = act_pool.tile([128, 2 * 1024], MMDT)    # heads (2m,2m+1) rows, tokens free
    kT = act_pool.tile([128, 2 * 1024], MMDT)
    qiT = act_pool.tile([128, 2 * 1024], F32)
    kiw = act_pool.tile([128, 1024], F32)        # rows 0-63 kiT, 64-67 wT logits
    kiw2 = act_pool.tile([128, 1024], F32)       # rows 64-127: copy of kiT (odd index heads)
    v_sb = act_pool.tile([128, 8 * 256], MMDT)    # [s-chunk sc] at cols sc*256, head cols inside
    w8 = act_pool.tile([128, 32], F32)           # softmax(x@Ww)/8, chunk i at cols 4i
    mask_tiles = [act_pool.tile([128, (i + 1) * 128], F32, name=f"mask{i}", tag=f"mask{i}") for i in range(NCHUNK)]

    # ---- Phase A: load x, build xT via PE transposes ----
    with tc.tile_pool(name="tp_ps", bufs=2, space="PSUM") as tp_ps, \
         tc.tile_pool(name="mm_ps", bufs=4, space="PSUM") as mm_ps:
      with tc.tile_pool(name="xscope", bufs=1) as xscope, \
           tc.tile_pool(name="xtok", bufs=2) as xtok_pool:
        xT = xscope.tile([128, 8 * 1024], F32)   # [d-chunk j] at cols j*1024, feature-major
        xTr = xscope.tile([128, 8 * 1024], MMDT)  # rounded shadow for fp32r matmuls
        wi_sb = xscope.tile([128, 8 * 324], F32)
        for j in range(8):
            nc.sync.dma_start(wi_sb[:, j * 324:(j + 1) * 324], wi_ap[j * 128:(j + 1) * 128, :])

        for i in range(NCHUNK):
            xt = xtok_pool.tile([128, 1024], F32, tag="xtok")
            nc.sync.dma_start(xt[:], x_ap[i * 128:(i + 1) * 128, :])
            for j in range(8):
                pt = tp_ps.tile([128, 128], F32, tag="tp")
                nc.tensor.transpose(pt[:], xt[:, j * 128:(j + 1) * 128], ident[:])
                nc.scalar.copy(xT[:, j * 1024 + i * 128: j * 1024 + (i + 1) * 128], pt[:])
        for j in range(8):
            nc.scalar.copy(xTr[:, j * 1024:(j + 1) * 1024], xT[:, j * 1024:(j + 1) * 1024])

        # ---- Phase B: projections (contract d over 8 chunks) ----
        # qT/kT/qiT: out [128 (2 heads x 64), t512] ; lhsT = W[:, m*128:+128]
        for (wsb, dst) in ((wq_sb, qT), (wk_sb, kT)):
            for m in range(2):
                for tg in range(2):
                    ps = mm_ps.tile([128, 512], F32, tag="mm")
                    for j in range(8):
                        nc.tensor.matmul(
                            ps[:],
                            wsb[:, j * 256 + m * 128: j * 256 + (m + 1) * 128],
                            xTr[:, j * 1024 + tg * 512: j * 1024 + (tg + 1) * 512],
                            start=(j == 0), stop=(j == 7))
                    nc.scalar.copy(dst[:, m * 1024 + tg * 512: m * 1024 + (tg + 1) * 512], ps[:])
        for m in range(2):  # qiT
            for tg in range(2):
                ps = mm_ps.tile([128, 512], F32, tag="mm")
                for j in range(8):
                    nc.tensor.matmul(
                        ps[:],
                        wi_sb[:, j * 324 + m * 128: j * 324 + (m + 1) * 128],
                        xT[:, j * 1024 + tg * 512: j * 1024 + (tg + 1) * 512],
                        start=(j == 0), stop=(j == 7))
                nc.scalar.copy(qiT[:, m * 1024 + tg * 512: m * 1024 + (tg + 1) * 512], ps[:])
        for tg in range(2):  # kiT + wT logits (68 cols of wi)
            ps = mm_ps.tile([128, 512], F32, tag="mm")
            for j in range(8):
                nc.tensor.matmul(
                    ps[0:68, :],
                    wi_sb[:, j * 324 + 256: j * 324 + 324],
                    xT[:, j * 1024 + tg * 512: j * 1024 + (tg + 1) * 512],
                    start=(j == 0), stop=(j == 7))
            nc.scalar.copy(kiw[0:68, tg * 512:(tg + 1) * 512], ps[0:68, :])
        nc.sync.dma_start(kiw2[64:128, :], kiw[0:64, :])
        # v natural layout: out [s128, 256]
        for sc in range(NCHUNK):
            ps = mm_ps.tile([128, 512], F32, tag="mm")
            for j in range(8):
                nc.tensor.matmul(
                    ps[:, 0:256],
                    xTr[:, j * 1024 + sc * 128: j * 1024 + (sc + 1) * 128],
                    wv_sb[:, j * 256:(j + 1) * 256],
                    start=(j == 0), stop=(j == 7))
            nc.scalar.copy(v_sb[:, sc * 256:(sc + 1) * 256], ps[:, 0:256])

        # w softmax per chunk: transpose wT logits [4, t128] -> [t128, 4]
        for i in range(NCHUNK):
            pw = tp_ps.tile([128, 128], F32, tag="tp")
            nc.tensor.transpose(pw[:, 0:4], kiw[64:68, i * 128:(i + 1) * 128], ident[64:68, 64:68])
            wexp = act_pool.tile([128, 4], F32, tag="wexp", bufs=2)
            wden = act_pool.tile([128, 1], F32, tag="wden", bufs=2)
            nc.scalar.activation(wexp[:], pw[:, 0:4], AF.Exp, accum_out=wden[:])
            wrec = act_pool.tile([128, 1], F32, tag="wrec", bufs=2)
            nc.vector.reciprocal(wrec[:], wden[:])
            nc.vector.tensor_scalar(w8[:, i * 4:(i + 1) * 4], wexp[:], wrec[:], 0.125,
                                    op0=ALU.mult, op1=ALU.mult)

      # ---- Phases C+D, interleaved per t-group: topk(tg+1) overlaps attention(tg) ----
      with tc.tile_pool(name="idx", bufs=3) as idx_pool, \
           tc.tile_pool(name="attn", bufs=1) as attn_pool, \
           tc.tile_pool(name="attn2", bufs=3) as attn2_pool, \
           tc.tile_pool(name="ctx_ps", bufs=2, space="PSUM") as ctx_ps:
            ctxT = attn_pool.tile([128, 2 * 1024], MMDT)  # [ck] at cols ck*1024

            def emit_idx(i):
                n_s = (i + 1) * 128
                work = idx_pool.tile([128, 1024], F32, name="work", tag="work")
                for h in range(HI):
                    m, r = h // 2, (h % 2) * 64
                    dst = work if h == 0 else idx_pool.tile([128, 1024], F32, name="aw", tag="aw")
                    for grp in range((n_s + 511) // 512):
                        ns0, ns1 = grp * 512, min(n_s, (grp + 1) * 512)
                        ps = mm_ps.tile([128, 512], F32, name="ps", tag="mm")
                        ki_rhs = kiw[0:64, ns0:ns1] if r == 0 else kiw2[64:128, ns0:ns1]
                        nc.tensor.matmul(
                            ps[:, 0:ns1 - ns0],
                            qiT[r:r + 64, m * 1024 + i * 128: m * 1024 + (i + 1) * 128],
                            ki_rhs,
                            start=True, stop=True)
                        nc.scalar.activation(dst[:, ns0:ns1], ps[:, 0:ns1 - ns0], AF.Relu,
                                             scale=w8[:, i * 4 + h: i * 4 + h + 1])
                    if h > 0:
                        nc.gpsimd.tensor_tensor(work[:, 0:n_s], work[:, 0:n_s], dst[:, 0:n_s], op=ALU.add)
                nc.gpsimd.tensor_tensor(work[:, i * 128:(i + 1) * 128],
                                        work[:, i * 128:(i + 1) * 128], causal[:], op=ALU.add)
                tmax = idx_pool.tile([128, 8], F32, name="tmax", tag="tmax")
                for _ in range(8):
                    nc.vector.max(tmax[:], work[:, 0:n_s])
                    nc.vector.match_replace(work[:, 0:n_s], tmax[:], work[:, 0:n_s], SENT)
                mk = mask_tiles[i]
                nc.vector.tensor_scalar(mk[:], work[:, 0:n_s], SENT, NEG,
                                        op0=ALU.not_equal, op1=ALU.mult)
                nc.gpsimd.tensor_tensor(mk[:, i * 128:(i + 1) * 128],
                                        mk[:, i * 128:(i + 1) * 128], causal[:], op=ALU.add)

            def emit_attn_tg(tg):
                i_lo, i_hi = tg * 4, tg * 4 + 4
                for h in range(HG):
                    m, r = h // 2, (h % 2) * 64
                    probT = [attn_pool.tile([128, 512], MMDT, name=f"probT{sc}", tag=f"probT{sc}", bufs=2) for sc in range(i_hi)]
                    for i in range(i_lo, i_hi):
                        n_s = (i + 1) * 128
                        sc_sb = attn2_pool.tile([128, 1024], F32, name="sc_sb", tag="sc")
                        for grp in range((n_s + 511) // 512):
                            ns0, ns1 = grp * 512, min(n_s, (grp + 1) * 512)
                            ps = mm_ps.tile([128, 512], F32, name="ps", tag="mm")
                            nc.tensor.matmul(
                                ps[:, 0:ns1 - ns0],
                                qT[r:r + 64, m * 1024 + i * 128: m * 1024 + (i + 1) * 128],
                                kT[r:r + 64, m * 1024 + ns0: m * 1024 + ns1],
                                start=True, stop=True)
                            nc.vector.tensor_tensor(sc_sb[:, ns0:ns1], ps[:, 0:ns1 - ns0],
                                                    mask_tiles[i][:, ns0:ns1], op=ALU.add)
                        scr = attn2_pool.tile([128, 1024], F32, name="scr", tag="scr")
                        den = attn2_pool.tile([128, 1], F32, name="den", tag="den")
                        nc.scalar.activation(scr[:, 0:n_s], sc_sb[:, 0:n_s], AF.Exp,
                                             scale=0.125, accum_out=den[:])
                        rec = attn2_pool.tile([128, 1], F32, name="rec", tag="rec")
                        nc.vector.reciprocal(rec[:], den[:])
                        nc.gpsimd.tensor_scalar(sc_sb[:, 0:n_s], scr[:, 0:n_s], rec[:], None,
                                                op0=ALU.mult)
                        for sc in range(i + 1):
                            pt = tp_ps.tile([128, 128], F32, name="pt", tag="tp")
                            nc.tensor.transpose(pt[:], sc_sb[:, sc * 128:(sc + 1) * 128], ident[:])
                            nc.scalar.copy(probT[sc][:, (i - i_lo) * 128:(i - i_lo + 1) * 128], pt[:])
                    pc = ctx_ps.tile([64, 512], F32, name="pc", tag="ctx")
                    n_sc = i_hi
                    for sc in range(n_sc):
                        off = max(sc - i_lo, 0) * 128
                        nc.tensor.matmul(
                            pc[:, off:512],
                            v_sb[:, sc * 256 + h * 64: sc * 256 + (h + 1) * 64],
                            probT[sc][:, off:512],
                            start=(sc == 0), stop=(sc == n_sc - 1))
                    ck, rr = h // 2, (h % 2) * 64
                    nc.scalar.copy(ctxT[rr:rr + 64, ck * 1024 + tg * 512: ck * 1024 + (tg + 1) * 512], pc[:])
                for i in range(i_lo, i_hi):
                    out_sb = attn2_pool.tile([128, 1024], F32, name="out_sb", tag="out")
                    for og in range(2):
                        ps = mm_ps.tile([128, 512], F32, name="ps", tag="mm")
                        for ck in range(2):
                            nc.tensor.matmul(
                                ps[:],
                                ctxT[:, ck * 1024 + i * 128: ck * 1024 + (i + 1) * 128],
                                wo_sb[:, ck * 1024 + og * 512: ck * 1024 + (og + 1) * 512],
                                start=(ck == 0), stop=(ck == 1))
                        nc.scalar.copy(out_sb[:, og * 512:(og + 1) * 512], ps[:])
                    nc.sync.dma_start(out_ap[i * 128:(i + 1) * 128, :], out_sb[:])

            for tg in range(2):
                for i in range(tg * 4, tg * 4 + 4):
                    emit_idx(i)
                emit_attn_tg(tg)

    stack.close()


def _build_nc(loop=0):
    nc = bacc.Bacc("TRN2")
    x = nc.dram_tensor("x", [T, D], F32, kind="ExternalInput")
    wq = nc.dram_tensor("wq", [D, 256], F32, kind="ExternalInput")
    wk = nc.dram_tensor("wk", [D, 256], F32, kind="ExternalInput")
    wv = nc.dram_tensor("wv", [D, 256], F32, kind="ExternalInput")
    wo = nc.dram_tensor("wo", [256, D], F32, kind="ExternalInput")
    wi = nc.dram_tensor("wi", [D, 324], F32, kind="ExternalInput")
    out = nc.dram_tensor("out", [T, D], F32, kind="ExternalOutput")
    with tile.TileContext(nc) as tc:
        if loop:
            with tc.For_i(0, loop, 1):
                build_kernel(tc, out.ap(), x.ap(), wq.ap(), wk.ap(), wv.ap(), wo.ap(), wi.ap())
        else:
            build_kernel(tc, out.ap(), x.ap(), wq.ap(), wk.ap(), wv.ap(), wo.ap(), wi.ap())
    nc.compile()
    return nc


def kernel(x, Wq, Wk, Wv, Wo, bo, Wqi, Wki, Ww, _trace=False):
    _install_neff_cache()
    x, Wq, Wk, Wv, Wo, bo, Wqi, Wki, Ww = (
        np.ascontiguousarray(np.asarray(a, np.float32))
        for a in (x, Wq, Wk, Wv, Wo, bo, Wqi, Wki, Ww))
    wi = np.ascontiguousarray(np.concatenate([Wqi, Wki, Ww], axis=1))
    nc = _build_nc()
    in_maps = []
    for b in range(B):
        for g in range(4):
            c = slice(g * 256, (g + 1) * 256)
            in_maps.append({
                "x": np.ascontiguousarray(x[b]),
                "wq": np.ascontiguousarray(Wq[:, c]),
                "wk": np.ascontiguousarray(Wk[:, c]),
                "wv": np.ascontiguousarray(Wv[:, c]),
                "wo": np.ascontiguousarray(Wo[c, :]),
                "wi": wi,
            })
    res = run_bass_kernel_spmd(nc, in_maps, core_ids=list(range(8)), trace=_trace)
    outs = [r["out"] for r in res.results]
    full = np.stack([sum(outs[b * 4:(b + 1) * 4]) + bo for b in range(B)], axis=0)
    full = full.astype(np.float32)
    if _trace:
        return full, res
    return full


def _make_in_maps(x, Wq, Wk, Wv, Wo, Wqi, Wki, Ww):
    wi = np.ascontiguousarray(np.concatenate([Wqi, Wki, Ww], axis=1))
    in_maps = []
    for b in range(B):
        for g in range(4):
            c = slice(g * 256, (g + 1) * 256)
            in_maps.append({
                "x": np.ascontiguousarray(x[b]),
                "wq": np.ascontiguousarray(Wq[:, c]),
                "wk": np.ascontiguousarray(Wk[:, c]),
                "wv": np.ascontiguousarray(Wv[:, c]),
                "wo": np.ascontiguousarray(Wo[c, :]),
                "wi": wi,
            })
    return in_maps


def bench_exec_ns(inputs, iters=10, loop=256):
    """Per-iteration device time: the kernel body loops `loop` times inside one
    NEFF; dispatch-overhead floor (loop=1 variant) is subtracted via the slope
    between two loop counts. Returns ns per kernel iteration."""
    lo = max(1, loop // 8)
    t_hi = _bench_exec_wall(inputs, iters, loop)
    t_lo = _bench_exec_wall(inputs, iters, lo)
    return (t_hi - t_lo) / (loop - lo) * 1e9


def _bench_exec_wall(inputs, iters, loop):
    import time

    import jax
    from jax.experimental.shard_map import shard_map
    from jax.sharding import Mesh, NamedSharding, PartitionSpec

    import concourse.bass2jax as b2j

    _install_neff_cache()
    b2j.install_neuronx_cc_hook()
    nc = _build_nc(loop=loop)
    ins = {k: np.ascontiguousarray(np.asarray(v, np.float32)) for k, v in inputs.items()}
    in_maps = _make_in_maps(ins["x"], ins["Wq"], ins["Wk"], ins["Wv"], ins["Wo"],
                            ins["Wqi"], ins["Wki"], ins["Ww"])

    partition_name = nc.partition_id_tensor.name if nc.partition_id_tensor else None
    in_names, out_names, out_avals, zero_outs = [], [], [], []
    for alloc in nc.m.functions[0].allocations:
        if not isinstance(alloc, mybir.MemoryLocationSet):
            continue
        name = alloc.memorylocations[0].name
        if alloc.kind == "ExternalInput":
            if name != partition_name:
                in_names.append(name)
        elif alloc.kind == "ExternalOutput":
            shape = tuple(alloc.tensor_shape)
            dtype = mybir.dt.np(alloc.dtype)
            out_names.append(name)
            out_avals.append(jax.core.ShapedArray(shape, dtype))
            zero_outs.append(np.zeros(shape, dtype))
    n_params = len(in_names)
    all_in_names = list(in_names) + list(out_names)
    if partition_name is not None:
        all_in_names.append(partition_name)

    def _body(*args):
        operands = list(args)
        if partition_name is not None:
            operands.append(b2j.partition_id_tensor())
        outs = b2j._bass_exec_p.bind(
            *operands,
            out_avals=tuple(out_avals),
            in_names=tuple(all_in_names),
            out_names=tuple(out_names),
            lowering_input_output_aliases=(),
            sim_require_finite=True,
            sim_require_nnan=True,
            nc=nc,
        )
        return tuple(outs)

    n_cores = len(in_maps)
    devices = jax.devices()[:n_cores]
    mesh = Mesh(np.asarray(devices), ("core",))
    in_specs = (PartitionSpec("core"),) * (n_params + len(out_names))
    out_specs = (PartitionSpec("core"),) * len(out_names)
    fn = jax.jit(shard_map(_body, mesh=mesh, in_specs=in_specs,
                           out_specs=out_specs, check_rep=False))
    sharding = NamedSharding(mesh, PartitionSpec("core"))
    dev_args = [
        jax.device_put(
            np.concatenate([np.asarray(in_maps[c][nm]) for c in range(n_cores)], axis=0),
            sharding)
        for nm in in_names
    ] + [
        jax.device_put(np.concatenate([z] * n_cores, axis=0), sharding)
        for z in zero_outs
    ]
    r = fn(*dev_args)
    jax.block_until_ready(r)
    times = []
    for _ in range(iters):
        t0 = time.perf_counter()
        r = fn(*dev_args)
        jax.block_until_ready(r)
        times.append(time.perf_counter() - t0)
    return min(times)


if __name__ == "__main__":
    rng = np.random.default_rng(0)
    ins = {
        "x": rng.standard_normal((B, T, D)).astype(np.float32),
        "Wq": (rng.standard_normal((D, D)) * 0.02).astype(np.float32),
        "Wk": (rng.standard_normal((D, D)) * 0.02).astype(np.float32),
        "Wv": (rng.standard_normal((D, D)) * 0.02).astype(np.float32),
        "Wo": (rng.standard_normal((D, D)) * 0.02).astype(np.float32),
        "bo": np.zeros(D, np.float32),
        "Wqi": (rng.standard_normal((D, HI * IHD)) * 0.02).astype(np.float32),
        "Wki": (rng.standard_normal((D, IHD)) * 0.02).astype(np.float32),
        "Ww": (rng.standard_normal((D, HI)) * 0.02).astype(np.float32),
    }
    out = kernel(**ins)
    print("out", out.shape, out.dtype, float(np.abs(out).max()))


# revision 31
# speedup vs baseline: 1.1024x; 1.0283x over previous
"""Bass/Trainium2 kernel for MultiHeadAttentionWithDSA (sparse attention with
lightning-indexer top-64 key selection), sharded over 8 NeuronCores.

Sharding: core = b*4 + g  (b in {0,1} batch, g in {0..3} head-group of 4 heads).
Each core computes a partial output  ctx_g @ Wo[g*256:(g+1)*256, :]  for its
batch; the host sums the 4 partials per batch and adds the bias.

All matmuls run in true fp32 (4 cycles/row) because the top-64 selection must
match the fp32 reference's ordering exactly at the boundaries.
"""

import numpy as np

import concourse.bacc as bacc
import concourse.bass as bass
import concourse.mybir as mybir
import concourse.tile as tile
from concourse import masks
from concourse.bass_utils import run_bass_kernel_spmd

F32 = mybir.dt.float32
F32R = mybir.dt.float32r
USE_F32R = True
MMDT = F32R if USE_F32R else F32
AF = mybir.ActivationFunctionType
ALU = mybir.AluOpType

B, T, D = 2, 1024, 1024
H, HD = 16, 64          # total heads, head dim
HG = 4                  # heads per core
HI, IHD = 4, 64         # index heads, index head dim
TOPK = 64
NCHUNK = T // 128       # 8 token chunks of 128
NEG = -3.0e30           # causal-invalid marker (additive mask value)
SENT = -1.0e30          # match_replace sentinel (distinct from NEG)

_NEFF_CACHE = "/var/tmp/bass-neff-cache"


def _install_neff_cache():
    """walrus compile output cache keyed on BIR hash (compiles are minutes)."""
    import hashlib
    import os
    import shutil

    import concourse.bass2jax as b2j

    if getattr(b2j, "_dsa_neff_cache_installed", False):
        return
    orig = b2j.compile_bir_kernel

    def cached(bir_json, tmpdir, neff_name="file.neff"):
        try:
            h = hashlib.sha256(
                bir_json if isinstance(bir_json, bytes) else bir_json.encode()
            ).hexdigest()[:24]
            os.makedirs(_NEFF_CACHE, exist_ok=True)
            hit = os.path.join(_NEFF_CACHE, h + ".neff")
            if os.path.exists(hit):
                dst = os.path.join(tmpdir, neff_name)
                shutil.copyfile(hit, dst)
                return dst
            neff = orig(bir_json, tmpdir, neff_name)
            shutil.copyfile(neff, hit + ".tmp")
            os.replace(hit + ".tmp", hit)
            return neff
        except OSError:
            return orig(bir_json, tmpdir, neff_name)

    b2j.compile_bir_kernel = cached
    b2j._dsa_neff_cache_installed = True


def build_kernel(tc, out_ap, x_ap, wq_ap, wk_ap, wv_ap, wo_ap, wi_ap):
    """Emit the per-core kernel. All APs are DRAM tensors:
    x [1024,1024], wq/wk/wv [1024,256], wo [256,1024],
    wi [1024,324] = concat(Wqi[1024,256], Wki[1024,64], Ww[1024,4]).
    out [1024,1024] partial (pre-bias, pre-reduction over head groups).
    """
    nc = tc.nc
    from contextlib import ExitStack
    stack = ExitStack()

    const_pool = stack.enter_context(tc.tile_pool(name="const", bufs=1))
    ident = const_pool.tile([128, 128], F32)
    masks.make_identity(nc, ident[:])
    causal = const_pool.tile([128, 128], F32)
    masks.make_causal_mask(nc, causal[:], mask_val=NEG)

    w_pool = stack.enter_context(tc.tile_pool(name="weights", bufs=1))
    wq_sb = w_pool.tile([128, 8 * 256], MMDT)
    wk_sb = w_pool.tile([128, 8 * 256], MMDT)
    wv_sb = w_pool.tile([128, 8 * 256], MMDT)
    wo_sb = w_pool.tile([128, 2 * 1024], MMDT)
    with tc.tile_pool(name="wload", bufs=2) as wload:
        for (ap_, dst_) in ((wq_ap, wq_sb), (wk_ap, wk_sb), (wv_ap, wv_sb)):
            for j in range(8):
                wt = wload.tile([128, 256], F32, name="wt", tag="wt")
                nc.sync.dma_start(wt[:], ap_[j * 128:(j + 1) * 128, :])
                nc.scalar.copy(dst_[:, j * 256:(j + 1) * 256], wt[:])
        for ck in range(2):
            wt2 = wload.tile([128, 1024], F32, name="wt2", tag="wt2")
            nc.sync.dma_start(wt2[:], wo_ap[ck * 128:(ck + 1) * 128, :])
            nc.scalar.copy(wo_sb[:, ck * 1024:(ck + 1) * 1024], wt2[:])

    act_pool = stack.enter_context(tc.tile_pool(name="acts", bufs=1))
    qT = act_pool.tile([128, 2 * 1024], MMDT)    # heads (2m,2m+1) rows, tokens free
    kT = act_pool.tile([128, 2 * 1024], MMDT)
    qiT = act_pool.tile([128, 2 * 1024], F32)
    kiw = act_pool.tile([128, 1024], F32)        # rows 0-63 kiT, 64-67 wT logits
    kiw2 = act_pool.tile([128, 1024], F32)       # rows 64-127: copy of kiT (odd index heads)
    v_sb = act_pool.tile([128, 8 * 256], MMDT)    # [s-chunk sc] at cols sc*256, head cols inside
    w8 = act_pool.tile([128, 32], F32)           # softmax(x@Ww)/8, chunk i at cols 4i
    mask_tiles = [act_pool.tile([128, (i + 1) * 128], F32, name=f"mask{i}", tag=f"mask{i}") for i in range(NCHUNK)]

    # ---- Phase A: load x, build xT via PE transposes ----
    with tc.tile_pool(name="tp_ps", bufs=2, space="PSUM") as tp_ps, \
         tc.tile_pool(name="mm_ps", bufs=4, space="PSUM") as mm_ps:
      with tc.tile_pool(name="xscope", bufs=1) as xscope, \
           tc.tile_pool(name="xtok", bufs=2) as xtok_pool:
        xT = xscope.tile([128, 8 * 1024], F32)   # [d-chunk j] at cols j*1024, feature-major
        xTr = xscope.tile([128, 8 * 1024], MMDT)  # rounded shadow for fp32r matmuls
        wi_sb = xscope.tile([128, 8 * 324], F32)
        for j in range(8):
            nc.sync.dma_start(wi_sb[:, j * 324:(j + 1) * 324], wi_ap[j * 128:(j + 1) * 128, :])

        for i in range(NCHUNK):
            xt = xtok_pool.tile([128, 1024], F32, tag="xtok")
            nc.sync.dma_start(xt[:], x_ap[i * 128:(i + 1) * 128, :])
            for j in range(8):
                pt = tp_ps.tile([128, 128], F32, tag="tp")
                nc.tensor.transpose(pt[:], xt[:, j * 128:(j + 1) * 128], ident[:])
                nc.scalar.copy(xT[:, j * 1024 + i * 128: j * 1024 + (i + 1) * 128], pt[:])
        for j in range(8):
            nc.scalar.copy(xTr[:, j * 1024:(j + 1) * 1024], xT[:, j * 1024:(j + 1) * 1024])

        # ---- Phase B: projections (contract d over 8 chunks) ----
        # qT/kT/qiT: out [128 (2 heads x 64), t512] ; lhsT = W[:, m*128:+128]
        for (wsb, dst) in ((wq_sb, qT), (wk_sb, kT)):
            for m in range(2):
                for tg in range(2):
                    ps = mm_ps.tile([128, 512], F32, tag="mm")
                    for j in range(8):
                        nc.tensor.matmul(
                            ps[:],
                            wsb[:, j * 256 + m * 128: j * 256 + (m + 1) * 128],
                            xTr[:, j * 1024 + tg * 512: j * 1024 + (tg + 1) * 512],
                            start=(j == 0), stop=(j == 7))
                    nc.scalar.copy(dst[:, m * 1024 + tg * 512: m * 1024 + (tg + 1) * 512], ps[:])
        for m in range(2):  # qiT
            for tg in range(2):
                ps = mm_ps.tile([128, 512], F32, tag="mm")
                for j in range(8):
                    nc.tensor.matmul(
                        ps[:],
                        wi_sb[:, j * 324 + m * 128: j * 324 + (m + 1) * 128],
                        xT[:, j * 1024 + tg * 512: j * 1024 + (tg + 1) * 512],
                        start=(j == 0), stop=(j == 7))
                nc.scalar.copy(qiT[:, m * 1024 + tg * 512: m * 1024 + (tg + 1) * 512], ps[:])
        for tg in range(2):  # kiT + wT logits (68 cols of wi)
            ps = mm_ps.tile([128, 512], F32, tag="mm")
            for j in range(8):
                nc.tensor.matmul(
                    ps[0:68, :],
                    wi_sb[:, j * 324 + 256: j * 324 + 324],
                    xT[:, j * 1024 + tg * 512: j * 1024 + (tg + 1) * 512],
                    start=(j == 0), stop=(j == 7))
            nc.scalar.copy(kiw[0:68, tg * 512:(tg + 1) * 512], ps[0:68, :])
        nc.sync.dma_start(kiw2[64:128, :], kiw[0:64, :])
        # v natural layout: out [s128, 256]
        for sc in range(NCHUNK):
            ps = mm_ps.tile([128, 512], F32, tag="mm")
            for j in range(8):
                nc.tensor.matmul(
                    ps[:, 0:256],
                    xTr[:, j * 1024 + sc * 128: j * 1024 + (sc + 1) * 128],
                    wv_sb[:, j * 256:(j + 1) * 256],
                    start=(j == 0), stop=(j == 7))
            nc.scalar.copy(v_sb[:, sc * 256:(sc + 1) * 256], ps[:, 0:256])

        # w softmax per chunk: transpose wT logits [4, t128] -> [t128, 4]
        for i in range(NCHUNK):
            pw = tp_ps.tile([128, 128], F32, tag="tp")
            nc.tensor.transpose(pw[:, 0:4], kiw[64:68, i * 128:(i + 1) * 128], ident[64:68, 64:68])
            wexp = act_pool.tile([128, 4], F32, tag="wexp", bufs=2)
            wden = act_pool.tile([128, 1], F32, tag="wden", bufs=2)
            nc.scalar.activation(wexp[:], pw[:, 0:4], AF.Exp, accum_out=wden[:])
            wrec = act_pool.tile([128, 1], F32, tag="wrec", bufs=2)
            nc.vector.reciprocal(wrec[:], wden[:])
            nc.vector.tensor_scalar(w8[:, i * 4:(i + 1) * 4], wexp[:], wrec[:], 0.125,
                                    op0=ALU.mult, op1=ALU.mult)

      # ---- Phases C+D, interleaved per t-group: topk(tg+1) overlaps attention(tg) ----
      with tc.tile_pool(name="idx", bufs=3) as idx_pool, \
           tc.tile_pool(name="attn", bufs=1) as attn_pool, \
           tc.tile_pool(name="attn2", bufs=3) as attn2_pool, \
           tc.tile_pool(name="ctx_ps", bufs=2, space="PSUM") as ctx_ps:
            ctxT = attn_pool.tile([128, 2 * 1024], MMDT)  # [ck] at cols ck*1024

            def emit_idx(i):
                n_s = (i + 1) * 128
                work = idx_pool.tile([128, 1024], F32, name="work", tag="work")
                for h in range(HI):
                    m, r = h // 2, (h % 2) * 64
                    dst = work if h == 0 else idx_pool.tile([128, 1024], F32, name="aw", tag="aw")
                    for grp in range((n_s + 511) // 512):
                        ns0, ns1 = grp * 512, min(n_s, (grp + 1) * 512)
                        ps = mm_ps.tile([128, 512], F32, name="ps", tag="mm")
                        ki_rhs = kiw[0:64, ns0:ns1] if r == 0 else kiw2[64:128, ns0:ns1]
                        nc.tensor.matmul(
                            ps[:, 0:ns1 - ns0],
                            qiT[r:r + 64, m * 1024 + i * 128: m * 1024 + (i + 1) * 128],
                            ki_rhs,
                            start=True, stop=True)
                        nc.scalar.activation(dst[:, ns0:ns1], ps[:, 0:ns1 - ns0], AF.Relu,
                                             scale=w8[:, i * 4 + h: i * 4 + h + 1])
                    if h > 0:
                        nc.gpsimd.tensor_tensor(work[:, 0:n_s], work[:, 0:n_s], dst[:, 0:n_s], op=ALU.add)
                nc.gpsimd.tensor_tensor(work[:, i * 128:(i + 1) * 128],
                                        work[:, i * 128:(i + 1) * 128], causal[:], op=ALU.add)
                tmax = idx_pool.tile([128, 8], F32, name="tmax", tag="tmax")
                for _ in range(8):
                    nc.vector.max(tmax[:], work[:, 0:n_s])
                    nc.vector.match_replace(work[:, 0:n_s], tmax[:], work[:, 0:n_s], SENT)
                mk = mask_tiles[i]
                nc.vector.tensor_scalar(mk[:], work[:, 0:n_s], SENT, NEG,
                                        op0=ALU.not_equal, op1=ALU.mult)
                nc.gpsimd.tensor_tensor(mk[:, i * 128:(i + 1) * 128],
                                        mk[:, i * 128:(i + 1) * 128], causal[:], op=ALU.add)

            def emit_attn_tg(tg):
                i_lo, i_hi = tg * 4, tg * 4 + 4
                for h in range(HG):
                    m, r = h // 2, (h % 2) * 64
                    probT = [attn_pool.tile([128, 512], MMDT, name=f"probT{sc}", tag=f"probT{sc}", bufs=2) for sc in range(i_hi)]
                    for i in range(i_lo, i_hi):
                        n_s = (i + 1) * 128
                        sc_sb = attn2_pool.tile([128, 1024], F32, name="sc_sb", tag="sc")
                        for grp in range((n_s + 511) // 512):
                            ns0, ns1 = grp * 512, min(n_s, (grp + 1) * 512)
                            ps = mm_ps.tile([128, 512], F32, name="ps", tag="mm")
                            nc.tensor.matmul(
                                ps[:, 0:ns1 - ns0],
                                qT[r:r + 64, m * 1024 + i * 128: m * 1024 + (i + 1) * 128],
                                kT[r:r + 64, m * 1024 + ns0: m * 1024 + ns1],
                                start=True, stop=True)
                            nc.vector.tensor_tensor(sc_sb[:, ns0:ns1], ps[:, 0:ns1 - ns0],
                                                    mask_tiles[i][:, ns0:ns1], op=ALU.add)
                        scr = attn2_pool.tile([128, 1024], F32, name="scr", tag="scr")
                        den = attn2_pool.tile([128, 1], F32, name="den", tag="den")
                        nc.scalar.activation(scr[:, 0:n_s], sc_sb[:, 0:n_s], AF.Exp,
                                             scale=0.125, accum_out=den[:])
                        rec = attn2_pool.tile([128, 1], F32, name="rec", tag="rec")
                        nc.vector.reciprocal(rec[:], den[:])
                        nc.gpsimd.tensor_scalar(sc_sb[:, 0:n_s], scr[:, 0:n_s], rec[:], None,
                                                op0=ALU.mult)
                        for sc in range(i + 1):
                            pt = tp_ps.tile([128, 128], F32, name="pt", tag="tp")
                            nc.tensor.transpose(pt[:], sc_sb[:, sc * 128:(sc + 1) * 128], ident[:])
                            nc.scalar.copy(probT[sc][:, (i - i_lo) * 128:(i - i_lo + 1) * 128], pt[:])
                    pc = ctx_ps.tile([64, 512], F32, name="pc", tag="ctx")
                    n_sc = i_hi
                    for sc in range(n_sc):
                        off = max(sc - i_lo, 0) * 128
                        nc.tensor.matmul(
                            pc[:, off:512],
                            v_sb[:, sc * 256 + h * 64: sc * 256 + (h + 1) * 64],
                            probT[sc][:, off:512],
                            start=(sc == 0), stop=(sc == n_sc - 1))
                    ck, rr = h // 2, (h % 2) * 64
                    nc.scalar.copy(ctxT[rr:rr + 64, ck * 1024 + tg * 512: ck * 1024 + (tg + 1) * 512], pc[:])
                for i in range(i_lo, i_hi):
                    out_sb = attn2_pool.tile([128, 1024], F32, name="out_sb", tag="out")
                    for og in range(2):
                        ps = mm_ps.tile([128, 512], F32, name="ps", tag="mm")
                        for ck in range(2):
                            nc.tensor.matmul(
                                ps[:],
                                ctxT[:, ck * 1024 + i * 128: ck * 1024 + (i + 1) * 128],
                                wo_sb[:, ck * 1024 + og * 512: ck * 1024 + (og + 1) * 512],
                                start=(ck == 0), stop=(ck == 1))
                        nc.scalar.copy(out_sb[:, og * 512:(og + 1) * 512], ps[:])
                    nc.sync.dma_start(out_ap[i * 128:(i + 1) * 128, :], out_sb[:])

            for i in range(NCHUNK):
                emit_idx(i)
            for tg in range(2):
                emit_attn_tg(tg)

    stack.close()


def _build_nc(loop=0):
    nc = bacc.Bacc("TRN2")
    x = nc.dram_tensor("x", [T, D], F32, kind="ExternalInput")
    wq = nc.dram_tensor("wq", [D, 256], F32, kind="ExternalInput")
    wk = nc.dram_tensor("wk", [D, 256], F32, kind="ExternalInput")
    wv = nc.dram_tensor("wv", [D, 256], F32, kind="ExternalInput")
    wo = nc.dram_tensor("wo", [256, D], F32, kind="ExternalInput")
    wi = nc.dram_tensor("wi", [D, 324], F32, kind="ExternalInput")
    out = nc.dram_tensor("out", [T, D], F32, kind="ExternalOutput")
    with tile.TileContext(nc) as tc:
        if loop:
            with tc.For_i(0, loop, 1):
                build_kernel(tc, out.ap(), x.ap(), wq.ap(), wk.ap(), wv.ap(), wo.ap(), wi.ap())
        else:
            build_kernel(tc, out.ap(), x.ap(), wq.ap(), wk.ap(), wv.ap(), wo.ap(), wi.ap())
    nc.compile()
    return nc


def kernel(x, Wq, Wk, Wv, Wo, bo, Wqi, Wki, Ww, _trace=False):
    _install_neff_cache()
    x, Wq, Wk, Wv, Wo, bo, Wqi, Wki, Ww = (
        np.ascontiguousarray(np.asarray(a, np.float32))
        for a in (x, Wq, Wk, Wv, Wo, bo, Wqi, Wki, Ww))
    wi = np.ascontiguousarray(np.concatenate([Wqi, Wki, Ww], axis=1))
    nc = _build_nc()
    in_maps = []
    for b in range(B):
        for g in range(4):
            c = slice(g * 256, (g + 1) * 256)
            in_maps.append({
                "x": np.ascontiguousarray(x[b]),
                "wq": np.ascontiguousarray(Wq[:, c]),
                "wk": np.ascontiguousarray(Wk[:, c]),
                "wv": np.ascontiguousarray(Wv[:, c]),
                "wo": np.ascontiguousarray(Wo[c, :]),
                "wi": wi,
            })
    res = run_bass_kernel_spmd(nc, in_maps, core_ids=list(range(8)), trace=_trace)
    outs = [r["out"] for r in res.results]
    full = np.stack([sum(outs[b * 4:(b + 1) * 4]) + bo for b in range(B)], axis=0)
    full = full.astype(np.float32)
    if _trace:
        return full, res
    return full


def _make_in_maps(x, Wq, Wk, Wv, Wo, Wqi, Wki, Ww):
    wi = np.ascontiguousarray(np.concatenate([Wqi, Wki, Ww], axis=1))
    in_maps = []
    for b in range(B):
        for g in range(4):
            c = slice(g * 256, (g + 1) * 256)
            in_maps.append({
                "x": np.ascontiguousarray(x[b]),
                "wq": np.ascontiguousarray(Wq[:, c]),
                "wk": np.ascontiguousarray(Wk[:, c]),
                "wv": np.ascontiguousarray(Wv[:, c]),
                "wo": np.ascontiguousarray(Wo[c, :]),
                "wi": wi,
            })
    return in_maps


def bench_exec_ns(inputs, iters=10, loop=256):
    """Per-iteration device time: the kernel body loops `loop` times inside one
    NEFF; dispatch-overhead floor (loop=1 variant) is subtracted via the slope
    between two loop counts. Returns ns per kernel iteration."""
    lo = max(1, loop // 8)
    t_hi = _bench_exec_wall(inputs, iters, loop)
    t_lo = _bench_exec_wall(inputs, iters, lo)
    return (t_hi - t_lo) / (loop - lo) * 1e9


def _bench_exec_wall(inputs, iters, loop):
    import time

    import jax
    from jax.experimental.shard_map import shard_map
    from jax.sharding import Mesh, NamedSharding, PartitionSpec

    import concourse.bass2jax as b2j

    _install_neff_cache()
    b2j.install_neuronx_cc_hook()
    nc = _build_nc(loop=loop)
    ins = {k: np.ascontiguousarray(np.asarray(v, np.float32)) for k, v in inputs.items()}
    in_maps = _make_in_maps(ins["x"], ins["Wq"], ins["Wk"], ins["Wv"], ins["Wo"],
                            ins["Wqi"], ins["Wki"], ins["Ww"])

    partition_name = nc.partition_id_tensor.name if nc.partition_id_tensor else None
    in_names, out_names, out_avals, zero_outs = [], [], [], []
    for alloc in nc.m.functions[0].allocations:
        if not isinstance(alloc, mybir.MemoryLocationSet):
            continue
        name = alloc.memorylocations[0].name
        if alloc.kind == "ExternalInput":
            if name != partition_name:
                in_names.append(name)
        elif alloc.kind == "ExternalOutput":
            shape = tuple(alloc.tensor_shape)
            dtype = mybir.dt.np(alloc.dtype)
            out_names.append(name)
            out_avals.append(jax.core.ShapedArray(shape, dtype))
            zero_outs.append(np.zeros(shape, dtype))
    n_params = len(in_names)
    all_in_names = list(in_names) + list(out_names)
    if partition_name is not None:
        all_in_names.append(partition_name)

    def _body(*args):
        operands = list(args)
        if partition_name is not None:
            operands.append(b2j.partition_id_tensor())
        outs = b2j._bass_exec_p.bind(
            *operands,
            out_avals=tuple(out_avals),
            in_names=tuple(all_in_names),
            out_names=tuple(out_names),
            lowering_input_output_aliases=(),
            sim_require_finite=True,
            sim_require_nnan=True,
            nc=nc,
        )
        return tuple(outs)

    n_cores = len(in_maps)
    devices = jax.devices()[:n_cores]
    mesh = Mesh(np.asarray(devices), ("core",))
    in_specs = (PartitionSpec("core"),) * (n_params + len(out_names))
    out_specs = (PartitionSpec("core"),) * len(out_names)
    fn = jax.jit(shard_map(_body, mesh=mesh, in_specs=in_specs,
                           out_specs=out_specs, check_rep=False))
    sharding = NamedSharding(mesh, PartitionSpec("core"))
    dev_args = [
        jax.device_put(
            np.concatenate([np.asarray(in_maps[c][nm]) for c in range(n_cores)], axis=0),
            sharding)
        for nm in in_names
    ] + [
        jax.device_put(np.concatenate([z] * n_cores, axis=0), sharding)
        for z in zero_outs
    ]
    r = fn(*dev_args)
    jax.block_until_ready(r)
    times = []
    for _ in range(iters):
        t0 = time.perf_counter()
        r = fn(*dev_args)
        jax.block_until_ready(r)
        times.append(time.perf_counter() - t0)
    return min(times)


if __name__ == "__main__":
    rng = np.random.default_rng(0)
    ins = {
        "x": rng.standard_normal((B, T, D)).astype(np.float32),
        "Wq": (rng.standard_normal((D, D)) * 0.02).astype(np.float32),
        "Wk": (rng.standard_normal((D, D)) * 0.02).astype(np.float32),
        "Wv": (rng.standard_normal((D, D)) * 0.02).astype(np.float32),
        "Wo": (rng.standard_normal((D, D)) * 0.02).astype(np.float32),
        "bo": np.zeros(D, np.float32),
        "Wqi": (rng.standard_normal((D, HI * IHD)) * 0.02).astype(np.float32),
        "Wki": (rng.standard_normal((D, IHD)) * 0.02).astype(np.float32),
        "Ww": (rng.standard_normal((D, HI)) * 0.02).astype(np.float32),
    }
    out = kernel(**ins)
    print("out", out.shape, out.dtype, float(np.abs(out).max()))


# revision 34
# speedup vs baseline: 1.1396x; 1.0337x over previous
"""Bass/Trainium2 kernel for MultiHeadAttentionWithDSA (sparse attention with
lightning-indexer top-64 key selection), sharded over 8 NeuronCores.

Sharding: core = b*4 + g  (b in {0,1} batch, g in {0..3} head-group of 4 heads).
Each core computes a partial output  ctx_g @ Wo[g*256:(g+1)*256, :]  for its
batch; the host sums the 4 partials per batch and adds the bias.

All matmuls run in true fp32 (4 cycles/row) because the top-64 selection must
match the fp32 reference's ordering exactly at the boundaries.
"""

import numpy as np

import concourse.bacc as bacc
import concourse.bass as bass
import concourse.mybir as mybir
import concourse.tile as tile
from concourse import masks
from concourse.bass_utils import run_bass_kernel_spmd

F32 = mybir.dt.float32
F32R = mybir.dt.float32r
USE_F32R = True
MMDT = F32R if USE_F32R else F32
AF = mybir.ActivationFunctionType
ALU = mybir.AluOpType

B, T, D = 2, 1024, 1024
H, HD = 16, 64          # total heads, head dim
HG = 4                  # heads per core
HI, IHD = 4, 64         # index heads, index head dim
TOPK = 64
NCHUNK = T // 128       # 8 token chunks of 128
NEG = -3.0e30           # causal-invalid marker (additive mask value)
SENT = -1.0e30          # match_replace sentinel (distinct from NEG)

_NEFF_CACHE = "/var/tmp/bass-neff-cache"


def _install_neff_cache():
    """walrus compile output cache keyed on BIR hash (compiles are minutes)."""
    import hashlib
    import os
    import shutil

    import concourse.bass2jax as b2j

    if getattr(b2j, "_dsa_neff_cache_installed", False):
        return
    orig = b2j.compile_bir_kernel

    def cached(bir_json, tmpdir, neff_name="file.neff"):
        try:
            h = hashlib.sha256(
                bir_json if isinstance(bir_json, bytes) else bir_json.encode()
            ).hexdigest()[:24]
            os.makedirs(_NEFF_CACHE, exist_ok=True)
            hit = os.path.join(_NEFF_CACHE, h + ".neff")
            if os.path.exists(hit):
                dst = os.path.join(tmpdir, neff_name)
                shutil.copyfile(hit, dst)
                return dst
            neff = orig(bir_json, tmpdir, neff_name)
            shutil.copyfile(neff, hit + ".tmp")
            os.replace(hit + ".tmp", hit)
            return neff
        except OSError:
            return orig(bir_json, tmpdir, neff_name)

    b2j.compile_bir_kernel = cached
    b2j._dsa_neff_cache_installed = True


def build_kernel(tc, out_ap, x_ap, wq_ap, wk_ap, wv_ap, wo_ap, wi_ap):
    """Emit the per-core kernel. All APs are DRAM tensors:
    x [1024,1024], wq/wk/wv [1024,256], wo [256,1024],
    wi [1024,324] = concat(Wqi[1024,256], Wki[1024,64], Ww[1024,4]).
    out [1024,1024] partial (pre-bias, pre-reduction over head groups).
    """
    nc = tc.nc
    from contextlib import ExitStack
    stack = ExitStack()

    const_pool = stack.enter_context(tc.tile_pool(name="const", bufs=1))
    ident = const_pool.tile([128, 128], F32)
    masks.make_identity(nc, ident[:])
    causal = const_pool.tile([128, 128], F32)
    masks.make_causal_mask(nc, causal[:], mask_val=NEG)

    w_pool = stack.enter_context(tc.tile_pool(name="weights", bufs=1))
    wq_sb = w_pool.tile([128, 8 * 256], MMDT)
    wk_sb = w_pool.tile([128, 8 * 256], MMDT)
    wv_sb = w_pool.tile([128, 8 * 256], MMDT)
    wo_sb = w_pool.tile([128, 2 * 1024], MMDT)
    with tc.tile_pool(name="wload", bufs=2) as wload:
        for (ap_, dst_) in ((wq_ap, wq_sb), (wk_ap, wk_sb), (wv_ap, wv_sb)):
            for j in range(8):
                wt = wload.tile([128, 256], F32, name="wt", tag="wt")
                nc.sync.dma_start(wt[:], ap_[j * 128:(j + 1) * 128, :])
                nc.scalar.copy(dst_[:, j * 256:(j + 1) * 256], wt[:])
        for ck in range(2):
            wt2 = wload.tile([128, 1024], F32, name="wt2", tag="wt2")
            nc.sync.dma_start(wt2[:], wo_ap[ck * 128:(ck + 1) * 128, :])
            nc.scalar.copy(wo_sb[:, ck * 1024:(ck + 1) * 1024], wt2[:])

    act_pool = stack.enter_context(tc.tile_pool(name="acts", bufs=1))
    qT = act_pool.tile([128, 2 * 1024], MMDT)    # heads (2m,2m+1) rows, tokens free
    kT = act_pool.tile([128, 2 * 1024], MMDT)
    qiT = act_pool.tile([128, 2 * 1024], F32)
    kiw = act_pool.tile([128, 1024], F32)        # rows 0-63 kiT, 64-67 wT logits
    kiw2 = act_pool.tile([128, 1024], F32)       # rows 64-127: copy of kiT (odd index heads)
    v_sb = act_pool.tile([128, 8 * 256], MMDT)    # [s-chunk sc] at cols sc*256, head cols inside
    w8 = act_pool.tile([128, 32], F32)           # softmax(x@Ww)/8, chunk i at cols 4i
    mask_tiles = [act_pool.tile([128, (i + 1) * 128], F32, name=f"mask{i}", tag=f"mask{i}") for i in range(NCHUNK)]

    # ---- Phase A: load x, build xT via PE transposes ----
    with tc.tile_pool(name="tp_ps", bufs=2, space="PSUM") as tp_ps, \
         tc.tile_pool(name="mm_ps", bufs=4, space="PSUM") as mm_ps:
      with tc.tile_pool(name="xscope", bufs=1) as xscope, \
           tc.tile_pool(name="xtok", bufs=2) as xtok_pool:
        xT = xscope.tile([128, 8 * 1024], F32)   # [d-chunk j] at cols j*1024, feature-major
        xTr = xscope.tile([128, 8 * 1024], MMDT)  # rounded shadow for fp32r matmuls
        wi_sb = xscope.tile([128, 8 * 324], F32)
        for j in range(8):
            nc.sync.dma_start(wi_sb[:, j * 324:(j + 1) * 324], wi_ap[j * 128:(j + 1) * 128, :])

        for i in range(NCHUNK):
            xt = xtok_pool.tile([128, 1024], F32, tag="xtok")
            nc.sync.dma_start(xt[:], x_ap[i * 128:(i + 1) * 128, :])
            for j in range(8):
                pt = tp_ps.tile([128, 128], F32, tag="tp")
                nc.tensor.transpose(pt[:], xt[:, j * 128:(j + 1) * 128], ident[:])
                nc.scalar.copy(xT[:, j * 1024 + i * 128: j * 1024 + (i + 1) * 128], pt[:])
        for j in range(8):
            nc.vector.tensor_copy(xTr[:, j * 1024:(j + 1) * 1024], xT[:, j * 1024:(j + 1) * 1024])

        # ---- Phase B: projections (contract d over 8 chunks) ----
        # qT/kT/qiT: out [128 (2 heads x 64), t512] ; lhsT = W[:, m*128:+128]
        for (wsb, dst) in ((wq_sb, qT), (wk_sb, kT)):
            for m in range(2):
                for tg in range(2):
                    ps = mm_ps.tile([128, 512], F32, tag="mm")
                    for j in range(8):
                        nc.tensor.matmul(
                            ps[:],
                            wsb[:, j * 256 + m * 128: j * 256 + (m + 1) * 128],
                            xTr[:, j * 1024 + tg * 512: j * 1024 + (tg + 1) * 512],
                            start=(j == 0), stop=(j == 7))
                    nc.scalar.copy(dst[:, m * 1024 + tg * 512: m * 1024 + (tg + 1) * 512], ps[:])
        for m in range(2):  # qiT
            for tg in range(2):
                ps = mm_ps.tile([128, 512], F32, tag="mm")
                for j in range(8):
                    nc.tensor.matmul(
                        ps[:],
                        wi_sb[:, j * 324 + m * 128: j * 324 + (m + 1) * 128],
                        xT[:, j * 1024 + tg * 512: j * 1024 + (tg + 1) * 512],
                        start=(j == 0), stop=(j == 7))
                nc.scalar.copy(qiT[:, m * 1024 + tg * 512: m * 1024 + (tg + 1) * 512], ps[:])
        for tg in range(2):  # kiT + wT logits (68 cols of wi)
            ps = mm_ps.tile([128, 512], F32, tag="mm")
            for j in range(8):
                nc.tensor.matmul(
                    ps[0:68, :],
                    wi_sb[:, j * 324 + 256: j * 324 + 324],
                    xT[:, j * 1024 + tg * 512: j * 1024 + (tg + 1) * 512],
                    start=(j == 0), stop=(j == 7))
            nc.scalar.copy(kiw[0:68, tg * 512:(tg + 1) * 512], ps[0:68, :])
        nc.sync.dma_start(kiw2[64:128, :], kiw[0:64, :])
        # v natural layout: out [s128, 256]
        for sc in range(NCHUNK):
            ps = mm_ps.tile([128, 512], F32, tag="mm")
            for j in range(8):
                nc.tensor.matmul(
                    ps[:, 0:256],
                    xTr[:, j * 1024 + sc * 128: j * 1024 + (sc + 1) * 128],
                    wv_sb[:, j * 256:(j + 1) * 256],
                    start=(j == 0), stop=(j == 7))
            nc.scalar.copy(v_sb[:, sc * 256:(sc + 1) * 256], ps[:, 0:256])

        # w softmax per chunk: transpose wT logits [4, t128] -> [t128, 4]
        for i in range(NCHUNK):
            pw = tp_ps.tile([128, 128], F32, tag="tp")
            nc.tensor.transpose(pw[:, 0:4], kiw[64:68, i * 128:(i + 1) * 128], ident[64:68, 64:68])
            wexp = act_pool.tile([128, 4], F32, tag="wexp", bufs=2)
            wden = act_pool.tile([128, 1], F32, tag="wden", bufs=2)
            nc.scalar.activation(wexp[:], pw[:, 0:4], AF.Exp, accum_out=wden[:])
            wrec = act_pool.tile([128, 1], F32, tag="wrec", bufs=2)
            nc.vector.reciprocal(wrec[:], wden[:])
            nc.vector.tensor_scalar(w8[:, i * 4:(i + 1) * 4], wexp[:], wrec[:], 0.125,
                                    op0=ALU.mult, op1=ALU.mult)

      # ---- Phases C+D, interleaved per t-group: topk(tg+1) overlaps attention(tg) ----
      with tc.tile_pool(name="idx", bufs=3) as idx_pool, \
           tc.tile_pool(name="attn", bufs=1) as attn_pool, \
           tc.tile_pool(name="attn2", bufs=3) as attn2_pool, \
           tc.tile_pool(name="ctx_ps", bufs=2, space="PSUM") as ctx_ps:
            ctxT = attn_pool.tile([128, 2 * 1024], MMDT)  # [ck] at cols ck*1024

            def emit_idx(i):
                n_s = (i + 1) * 128
                work = idx_pool.tile([128, 1024], F32, name="work", tag="work")
                for h in range(HI):
                    m, r = h // 2, (h % 2) * 64
                    dst = work if h == 0 else idx_pool.tile([128, 1024], F32, name="aw", tag="aw")
                    for grp in range((n_s + 511) // 512):
                        ns0, ns1 = grp * 512, min(n_s, (grp + 1) * 512)
                        ps = mm_ps.tile([128, 512], F32, name="ps", tag="mm")
                        ki_rhs = kiw[0:64, ns0:ns1] if r == 0 else kiw2[64:128, ns0:ns1]
                        nc.tensor.matmul(
                            ps[:, 0:ns1 - ns0],
                            qiT[r:r + 64, m * 1024 + i * 128: m * 1024 + (i + 1) * 128],
                            ki_rhs,
                            start=True, stop=True)
                        nc.scalar.activation(dst[:, ns0:ns1], ps[:, 0:ns1 - ns0], AF.Relu,
                                             scale=w8[:, i * 4 + h: i * 4 + h + 1])
                    if h > 0:
                        nc.gpsimd.tensor_tensor(work[:, 0:n_s], work[:, 0:n_s], dst[:, 0:n_s], op=ALU.add)
                nc.gpsimd.tensor_tensor(work[:, i * 128:(i + 1) * 128],
                                        work[:, i * 128:(i + 1) * 128], causal[:], op=ALU.add)
                tmax = idx_pool.tile([128, 8], F32, name="tmax", tag="tmax")
                for _ in range(8):
                    nc.vector.max(tmax[:], work[:, 0:n_s])
                    nc.vector.match_replace(work[:, 0:n_s], tmax[:], work[:, 0:n_s], SENT)
                mk = mask_tiles[i]
                nc.vector.tensor_scalar(mk[:], work[:, 0:n_s], SENT, NEG,
                                        op0=ALU.not_equal, op1=ALU.mult)
                nc.gpsimd.tensor_tensor(mk[:, i * 128:(i + 1) * 128],
                                        mk[:, i * 128:(i + 1) * 128], causal[:], op=ALU.add)

            def emit_attn_tg(tg):
                i_lo, i_hi = tg * 4, tg * 4 + 4
                for h in range(HG):
                    m, r = h // 2, (h % 2) * 64
                    probTall = attn_pool.tile([128, NCHUNK * 512], MMDT, name="probTall", tag="probTall", bufs=2)
                    for i in range(i_lo, i_hi):
                        n_s = (i + 1) * 128
                        sc_sb = attn2_pool.tile([128, 1024], F32, name="sc_sb", tag="sc")
                        for grp in range((n_s + 511) // 512):
                            ns0, ns1 = grp * 512, min(n_s, (grp + 1) * 512)
                            ps = mm_ps.tile([128, 512], F32, name="ps", tag="mm")
                            nc.tensor.matmul(
                                ps[:, 0:ns1 - ns0],
                                qT[r:r + 64, m * 1024 + i * 128: m * 1024 + (i + 1) * 128],
                                kT[r:r + 64, m * 1024 + ns0: m * 1024 + ns1],
                                start=True, stop=True)
                            nc.vector.tensor_tensor(sc_sb[:, ns0:ns1], ps[:, 0:ns1 - ns0],
                                                    mask_tiles[i][:, ns0:ns1], op=ALU.add)
                        scr = attn2_pool.tile([128, 1024], F32, name="scr", tag="scr")
                        den = attn2_pool.tile([128, 1], F32, name="den", tag="den")
                        nc.scalar.activation(scr[:, 0:n_s], sc_sb[:, 0:n_s], AF.Exp,
                                             scale=0.125, accum_out=den[:])
                        rec = attn2_pool.tile([128, 1], F32, name="rec", tag="rec")
                        nc.vector.reciprocal(rec[:], den[:])
                        nc.gpsimd.tensor_scalar(sc_sb[:, 0:n_s], scr[:, 0:n_s], rec[:], None,
                                                op0=ALU.mult)
                        toff = (i - i_lo) * 128
                        for bi in range((i + 4) // 4):
                            cnt = min(i + 1, bi * 4 + 4) - bi * 4
                            pt = tp_ps.tile([128, 512], F32, name="pt", tag="tp")
                            for q in range(cnt):
                                sc = bi * 4 + q
                                nc.tensor.matmul(pt[:, q * 128:(q + 1) * 128],
                                                 sc_sb[:, sc * 128:(sc + 1) * 128], ident[:],
                                                 is_transpose=True,
                                                 start=(q == 0), stop=(q == cnt - 1))
                            if cnt > 1:
                                base = bi * 4 * 512
                                dst = probTall[:, base: base + cnt * 512]
                                dst = dst.rearrange("p (c q) -> p c q", q=512)[:, :, toff:toff + 128]
                                srcv = pt[:, 0:cnt * 128].rearrange("p (c q) -> p c q", q=128)
                            else:
                                dst = probTall[:, bi * 4 * 512 + toff: bi * 4 * 512 + toff + 128]
                                srcv = pt[:, 0:128]
                            nc.scalar.copy(dst, srcv)
                    pc = ctx_ps.tile([64, 512], F32, name="pc", tag="ctx")
                    n_sc = i_hi
                    for sc in range(n_sc):
                        off = max(sc - i_lo, 0) * 128
                        nc.tensor.matmul(
                            pc[:, off:512],
                            v_sb[:, sc * 256 + h * 64: sc * 256 + (h + 1) * 64],
                            probTall[:, sc * 512 + off: sc * 512 + 512],
                            start=(sc == 0), stop=(sc == n_sc - 1))
                    ck, rr = h // 2, (h % 2) * 64
                    nc.scalar.copy(ctxT[rr:rr + 64, ck * 1024 + tg * 512: ck * 1024 + (tg + 1) * 512], pc[:])
                for i in range(i_lo, i_hi):
                    out_sb = attn2_pool.tile([128, 1024], F32, name="out_sb", tag="out")
                    for og in range(2):
                        ps = mm_ps.tile([128, 512], F32, name="ps", tag="mm")
                        for ck in range(2):
                            nc.tensor.matmul(
                                ps[:],
                                ctxT[:, ck * 1024 + i * 128: ck * 1024 + (i + 1) * 128],
                                wo_sb[:, ck * 1024 + og * 512: ck * 1024 + (og + 1) * 512],
                                start=(ck == 0), stop=(ck == 1))
                        nc.scalar.copy(out_sb[:, og * 512:(og + 1) * 512], ps[:])
                    nc.sync.dma_start(out_ap[i * 128:(i + 1) * 128, :], out_sb[:])

            for i in range(NCHUNK):
                emit_idx(i)
            for tg in range(2):
                emit_attn_tg(tg)

    stack.close()


def _build_nc(loop=0):
    nc = bacc.Bacc("TRN2")
    x = nc.dram_tensor("x", [T, D], F32, kind="ExternalInput")
    wq = nc.dram_tensor("wq", [D, 256], F32, kind="ExternalInput")
    wk = nc.dram_tensor("wk", [D, 256], F32, kind="ExternalInput")
    wv = nc.dram_tensor("wv", [D, 256], F32, kind="ExternalInput")
    wo = nc.dram_tensor("wo", [256, D], F32, kind="ExternalInput")
    wi = nc.dram_tensor("wi", [D, 324], F32, kind="ExternalInput")
    out = nc.dram_tensor("out", [T, D], F32, kind="ExternalOutput")
    with tile.TileContext(nc) as tc:
        if loop:
            with tc.For_i(0, loop, 1):
                build_kernel(tc, out.ap(), x.ap(), wq.ap(), wk.ap(), wv.ap(), wo.ap(), wi.ap())
        else:
            build_kernel(tc, out.ap(), x.ap(), wq.ap(), wk.ap(), wv.ap(), wo.ap(), wi.ap())
    nc.compile()
    return nc


def kernel(x, Wq, Wk, Wv, Wo, bo, Wqi, Wki, Ww, _trace=False):
    _install_neff_cache()
    x, Wq, Wk, Wv, Wo, bo, Wqi, Wki, Ww = (
        np.ascontiguousarray(np.asarray(a, np.float32))
        for a in (x, Wq, Wk, Wv, Wo, bo, Wqi, Wki, Ww))
    wi = np.ascontiguousarray(np.concatenate([Wqi, Wki, Ww], axis=1))
    nc = _build_nc()
    in_maps = []
    for b in range(B):
        for g in range(4):
            c = slice(g * 256, (g + 1) * 256)
            in_maps.append({
                "x": np.ascontiguousarray(x[b]),
                "wq": np.ascontiguousarray(Wq[:, c]),
                "wk": np.ascontiguousarray(Wk[:, c]),
                "wv": np.ascontiguousarray(Wv[:, c]),
                "wo": np.ascontiguousarray(Wo[c, :]),
                "wi": wi,
            })
    res = run_bass_kernel_spmd(nc, in_maps, core_ids=list(range(8)), trace=_trace)
    outs = [r["out"] for r in res.results]
    full = np.stack([sum(outs[b * 4:(b + 1) * 4]) + bo for b in range(B)], axis=0)
    full = full.astype(np.float32)
    if _trace:
        return full, res
    return full


def _make_in_maps(x, Wq, Wk, Wv, Wo, Wqi, Wki, Ww):
    wi = np.ascontiguousarray(np.concatenate([Wqi, Wki, Ww], axis=1))
    in_maps = []
    for b in range(B):
        for g in range(4):
            c = slice(g * 256, (g + 1) * 256)
            in_maps.append({
                "x": np.ascontiguousarray(x[b]),
                "wq": np.ascontiguousarray(Wq[:, c]),
                "wk": np.ascontiguousarray(Wk[:, c]),
                "wv": np.ascontiguousarray(Wv[:, c]),
                "wo": np.ascontiguousarray(Wo[c, :]),
                "wi": wi,
            })
    return in_maps


def bench_exec_ns(inputs, iters=10, loop=256):
    """Per-iteration device time: the kernel body loops `loop` times inside one
    NEFF; dispatch-overhead floor (loop=1 variant) is subtracted via the slope
    between two loop counts. Returns ns per kernel iteration."""
    lo = max(1, loop // 8)
    t_hi = _bench_exec_wall(inputs, iters, loop)
    t_lo = _bench_exec_wall(inputs, iters, lo)
    return (t_hi - t_lo) / (loop - lo) * 1e9


def _bench_exec_wall(inputs, iters, loop):
    import time

    import jax
    from jax.experimental.shard_map import shard_map
    from jax.sharding import Mesh, NamedSharding, PartitionSpec

    import concourse.bass2jax as b2j

    _install_neff_cache()
    b2j.install_neuronx_cc_hook()
    nc = _build_nc(loop=loop)
    ins = {k: np.ascontiguousarray(np.asarray(v, np.float32)) for k, v in inputs.items()}
    in_maps = _make_in_maps(ins["x"], ins["Wq"], ins["Wk"], ins["Wv"], ins["Wo"],
                            ins["Wqi"], ins["Wki"], ins["Ww"])

    partition_name = nc.partition_id_tensor.name if nc.partition_id_tensor else None
    in_names, out_names, out_avals, zero_outs = [], [], [], []
    for alloc in nc.m.functions[0].allocations:
        if not isinstance(alloc, mybir.MemoryLocationSet):
            continue
        name = alloc.memorylocations[0].name
        if alloc.kind == "ExternalInput":
            if name != partition_name:
                in_names.append(name)
        elif alloc.kind == "ExternalOutput":
            shape = tuple(alloc.tensor_shape)
            dtype = mybir.dt.np(alloc.dtype)
            out_names.append(name)
            out_avals.append(jax.core.ShapedArray(shape, dtype))
            zero_outs.append(np.zeros(shape, dtype))
    n_params = len(in_names)
    all_in_names = list(in_names) + list(out_names)
    if partition_name is not None:
        all_in_names.append(partition_name)

    def _body(*args):
        operands = list(args)
        if partition_name is not None:
            operands.append(b2j.partition_id_tensor())
        outs = b2j._bass_exec_p.bind(
            *operands,
            out_avals=tuple(out_avals),
            in_names=tuple(all_in_names),
            out_names=tuple(out_names),
            lowering_input_output_aliases=(),
            sim_require_finite=True,
            sim_require_nnan=True,
            nc=nc,
        )
        return tuple(outs)

    n_cores = len(in_maps)
    devices = jax.devices()[:n_cores]
    mesh = Mesh(np.asarray(devices), ("core",))
    in_specs = (PartitionSpec("core"),) * (n_params + len(out_names))
    out_specs = (PartitionSpec("core"),) * len(out_names)
    fn = jax.jit(shard_map(_body, mesh=mesh, in_specs=in_specs,
                           out_specs=out_specs, check_rep=False))
    sharding = NamedSharding(mesh, PartitionSpec("core"))
    dev_args = [
        jax.device_put(
            np.concatenate([np.asarray(in_maps[c][nm]) for c in range(n_cores)], axis=0),
            sharding)
        for nm in in_names
    ] + [
        jax.device_put(np.concatenate([z] * n_cores, axis=0), sharding)
        for z in zero_outs
    ]
    r = fn(*dev_args)
    jax.block_until_ready(r)
    times = []
    for _ in range(iters):
        t0 = time.perf_counter()
        r = fn(*dev_args)
        jax.block_until_ready(r)
        times.append(time.perf_counter() - t0)
    return min(times)


if __name__ == "__main__":
    rng = np.random.default_rng(0)
    ins = {
        "x": rng.standard_normal((B, T, D)).astype(np.float32),
        "Wq": (rng.standard_normal((D, D)) * 0.02).astype(np.float32),
        "Wk": (rng.standard_normal((D, D)) * 0.02).astype(np.float32),
        "Wv": (rng.standard_normal((D, D)) * 0.02).astype(np.float32),
        "Wo": (rng.standard_normal((D, D)) * 0.02).astype(np.float32),
        "bo": np.zeros(D, np.float32),
        "Wqi": (rng.standard_normal((D, HI * IHD)) * 0.02).astype(np.float32),
        "Wki": (rng.standard_normal((D, IHD)) * 0.02).astype(np.float32),
        "Ww": (rng.standard_normal((D, HI)) * 0.02).astype(np.float32),
    }
    out = kernel(**ins)
    print("out", out.shape, out.dtype, float(np.abs(out).max()))


# revision 35
# speedup vs baseline: 1.1559x; 1.0144x over previous
"""Bass/Trainium2 kernel for MultiHeadAttentionWithDSA (sparse attention with
lightning-indexer top-64 key selection), sharded over 8 NeuronCores.

Sharding: core = b*4 + g  (b in {0,1} batch, g in {0..3} head-group of 4 heads).
Each core computes a partial output  ctx_g @ Wo[g*256:(g+1)*256, :]  for its
batch; the host sums the 4 partials per batch and adds the bias.

All matmuls run in true fp32 (4 cycles/row) because the top-64 selection must
match the fp32 reference's ordering exactly at the boundaries.
"""

import numpy as np

import concourse.bacc as bacc
import concourse.bass as bass
import concourse.mybir as mybir
import concourse.tile as tile
from concourse import masks
from concourse.bass_utils import run_bass_kernel_spmd

F32 = mybir.dt.float32
F32R = mybir.dt.float32r
USE_F32R = True
MMDT = F32R if USE_F32R else F32
AF = mybir.ActivationFunctionType
ALU = mybir.AluOpType

B, T, D = 2, 1024, 1024
H, HD = 16, 64          # total heads, head dim
HG = 4                  # heads per core
HI, IHD = 4, 64         # index heads, index head dim
TOPK = 64
NCHUNK = T // 128       # 8 token chunks of 128
NEG = -3.0e30           # causal-invalid marker (additive mask value)
SENT = -1.0e30          # match_replace sentinel (distinct from NEG)

_NEFF_CACHE = "/var/tmp/bass-neff-cache"


def _install_neff_cache():
    """walrus compile output cache keyed on BIR hash (compiles are minutes)."""
    import hashlib
    import os
    import shutil

    import concourse.bass2jax as b2j

    if getattr(b2j, "_dsa_neff_cache_installed", False):
        return
    orig = b2j.compile_bir_kernel

    def cached(bir_json, tmpdir, neff_name="file.neff"):
        try:
            h = hashlib.sha256(
                bir_json if isinstance(bir_json, bytes) else bir_json.encode()
            ).hexdigest()[:24]
            os.makedirs(_NEFF_CACHE, exist_ok=True)
            hit = os.path.join(_NEFF_CACHE, h + ".neff")
            if os.path.exists(hit):
                dst = os.path.join(tmpdir, neff_name)
                shutil.copyfile(hit, dst)
                return dst
            neff = orig(bir_json, tmpdir, neff_name)
            shutil.copyfile(neff, hit + ".tmp")
            os.replace(hit + ".tmp", hit)
            return neff
        except OSError:
            return orig(bir_json, tmpdir, neff_name)

    b2j.compile_bir_kernel = cached
    b2j._dsa_neff_cache_installed = True


def build_kernel(tc, out_ap, x_ap, wq_ap, wk_ap, wv_ap, wo_ap, wi_ap):
    """Emit the per-core kernel. All APs are DRAM tensors:
    x [1024,1024], wq/wk/wv [1024,256], wo [256,1024],
    wi [1024,324] = concat(Wqi[1024,256], Wki[1024,64], Ww[1024,4]).
    out [1024,1024] partial (pre-bias, pre-reduction over head groups).
    """
    nc = tc.nc
    from contextlib import ExitStack
    stack = ExitStack()

    const_pool = stack.enter_context(tc.tile_pool(name="const", bufs=1))
    ident = const_pool.tile([128, 128], F32)
    masks.make_identity(nc, ident[:])
    causal = const_pool.tile([128, 128], F32)
    masks.make_causal_mask(nc, causal[:], mask_val=NEG)

    w_pool = stack.enter_context(tc.tile_pool(name="weights", bufs=1))
    wq_sb = w_pool.tile([128, 8 * 256], MMDT)
    wk_sb = w_pool.tile([128, 8 * 256], MMDT)
    wv_sb = w_pool.tile([128, 8 * 256], MMDT)
    wo_sb = w_pool.tile([128, 2 * 1024], MMDT)
    with tc.tile_pool(name="wload", bufs=2) as wload:
        for (ap_, dst_) in ((wq_ap, wq_sb), (wk_ap, wk_sb), (wv_ap, wv_sb)):
            for j in range(8):
                wt = wload.tile([128, 256], F32, name="wt", tag="wt")
                nc.sync.dma_start(wt[:], ap_[j * 128:(j + 1) * 128, :])
                nc.scalar.copy(dst_[:, j * 256:(j + 1) * 256], wt[:])
        for ck in range(2):
            wt2 = wload.tile([128, 1024], F32, name="wt2", tag="wt2")
            nc.sync.dma_start(wt2[:], wo_ap[ck * 128:(ck + 1) * 128, :])
            nc.scalar.copy(wo_sb[:, ck * 1024:(ck + 1) * 1024], wt2[:])

    act_pool = stack.enter_context(tc.tile_pool(name="acts", bufs=1))
    qT = act_pool.tile([128, 2 * 1024], MMDT)    # heads (2m,2m+1) rows, tokens free
    kT = act_pool.tile([128, 2 * 1024], MMDT)
    qiT = act_pool.tile([128, 2 * 1024], F32)
    kiw = act_pool.tile([128, 1024], F32)        # rows 0-63 kiT, 64-67 wT logits
    kiw2 = act_pool.tile([128, 1024], F32)       # rows 64-127: copy of kiT (odd index heads)
    v_sb = act_pool.tile([128, 8 * 256], MMDT)    # [s-chunk sc] at cols sc*256, head cols inside
    w8 = act_pool.tile([128, 32], F32)           # softmax(x@Ww)/8, chunk i at cols 4i
    mask_tiles = [act_pool.tile([128, (i + 1) * 128], F32, name=f"mask{i}", tag=f"mask{i}") for i in range(NCHUNK)]

    # ---- Phase A: load x, build xT via PE transposes ----
    with tc.tile_pool(name="tp_ps", bufs=2, space="PSUM") as tp_ps, \
         tc.tile_pool(name="mm_ps", bufs=4, space="PSUM") as mm_ps:
      with tc.tile_pool(name="xscope", bufs=1) as xscope, \
           tc.tile_pool(name="xtok", bufs=2) as xtok_pool:
        xT = xscope.tile([128, 8 * 1024], F32)   # [d-chunk j] at cols j*1024, feature-major
        xTr = xscope.tile([128, 8 * 1024], MMDT)  # rounded shadow for fp32r matmuls
        wi_sb = xscope.tile([128, 8 * 324], F32)
        for j in range(8):
            nc.sync.dma_start(wi_sb[:, j * 324:(j + 1) * 324], wi_ap[j * 128:(j + 1) * 128, :])

        for i in range(NCHUNK):
            xt = xtok_pool.tile([128, 1024], F32, tag="xtok")
            nc.sync.dma_start(xt[:], x_ap[i * 128:(i + 1) * 128, :])
            for bj in range(2):
                pt = tp_ps.tile([128, 512], F32, name="pt", tag="tp")
                for q in range(4):
                    j = bj * 4 + q
                    nc.tensor.matmul(pt[:, q * 128:(q + 1) * 128],
                                     xt[:, j * 128:(j + 1) * 128], ident[:],
                                     is_transpose=True, start=(q == 0), stop=(q == 3))
                dst = xT[:, bj * 4096: (bj + 1) * 4096]
                dst = dst.rearrange("p (c q) -> p c q", q=1024)[:, :, i * 128:(i + 1) * 128]
                nc.scalar.copy(dst, pt[:].rearrange("p (c q) -> p c q", q=128))
        for j in range(8):
            nc.vector.tensor_copy(xTr[:, j * 1024:(j + 1) * 1024], xT[:, j * 1024:(j + 1) * 1024])

        # ---- Phase B: projections (contract d over 8 chunks) ----
        # qT/kT/qiT: out [128 (2 heads x 64), t512] ; lhsT = W[:, m*128:+128]
        for (wsb, dst) in ((wq_sb, qT), (wk_sb, kT)):
            for m in range(2):
                for tg in range(2):
                    ps = mm_ps.tile([128, 512], F32, tag="mm")
                    for j in range(8):
                        nc.tensor.matmul(
                            ps[:],
                            wsb[:, j * 256 + m * 128: j * 256 + (m + 1) * 128],
                            xTr[:, j * 1024 + tg * 512: j * 1024 + (tg + 1) * 512],
                            start=(j == 0), stop=(j == 7))
                    nc.scalar.copy(dst[:, m * 1024 + tg * 512: m * 1024 + (tg + 1) * 512], ps[:])
        for m in range(2):  # qiT
            for tg in range(2):
                ps = mm_ps.tile([128, 512], F32, tag="mm")
                for j in range(8):
                    nc.tensor.matmul(
                        ps[:],
                        wi_sb[:, j * 324 + m * 128: j * 324 + (m + 1) * 128],
                        xT[:, j * 1024 + tg * 512: j * 1024 + (tg + 1) * 512],
                        start=(j == 0), stop=(j == 7))
                nc.scalar.copy(qiT[:, m * 1024 + tg * 512: m * 1024 + (tg + 1) * 512], ps[:])
        for tg in range(2):  # kiT + wT logits (68 cols of wi)
            ps = mm_ps.tile([128, 512], F32, tag="mm")
            for j in range(8):
                nc.tensor.matmul(
                    ps[0:68, :],
                    wi_sb[:, j * 324 + 256: j * 324 + 324],
                    xT[:, j * 1024 + tg * 512: j * 1024 + (tg + 1) * 512],
                    start=(j == 0), stop=(j == 7))
            nc.scalar.copy(kiw[0:68, tg * 512:(tg + 1) * 512], ps[0:68, :])
        nc.sync.dma_start(kiw2[64:128, :], kiw[0:64, :])
        # v natural layout: out [s128, 256]
        for sc in range(NCHUNK):
            ps = mm_ps.tile([128, 512], F32, tag="mm")
            for j in range(8):
                nc.tensor.matmul(
                    ps[:, 0:256],
                    xTr[:, j * 1024 + sc * 128: j * 1024 + (sc + 1) * 128],
                    wv_sb[:, j * 256:(j + 1) * 256],
                    start=(j == 0), stop=(j == 7))
            nc.scalar.copy(v_sb[:, sc * 256:(sc + 1) * 256], ps[:, 0:256])

        # w softmax per chunk: transpose wT logits [4, t128] -> [t128, 4]
        for i in range(NCHUNK):
            pw = tp_ps.tile([128, 128], F32, tag="tp")
            nc.tensor.transpose(pw[:, 0:4], kiw[64:68, i * 128:(i + 1) * 128], ident[64:68, 64:68])
            wexp = act_pool.tile([128, 4], F32, tag="wexp", bufs=2)
            wden = act_pool.tile([128, 1], F32, tag="wden", bufs=2)
            nc.scalar.activation(wexp[:], pw[:, 0:4], AF.Exp, accum_out=wden[:])
            wrec = act_pool.tile([128, 1], F32, tag="wrec", bufs=2)
            nc.vector.reciprocal(wrec[:], wden[:])
            nc.vector.tensor_scalar(w8[:, i * 4:(i + 1) * 4], wexp[:], wrec[:], 0.125,
                                    op0=ALU.mult, op1=ALU.mult)

      # ---- Phases C+D, interleaved per t-group: topk(tg+1) overlaps attention(tg) ----
      with tc.tile_pool(name="idx", bufs=3) as idx_pool, \
           tc.tile_pool(name="attn", bufs=1) as attn_pool, \
           tc.tile_pool(name="attn2", bufs=3) as attn2_pool, \
           tc.tile_pool(name="ctx_ps", bufs=2, space="PSUM") as ctx_ps:
            ctxT = attn_pool.tile([128, 2 * 1024], MMDT)  # [ck] at cols ck*1024

            def emit_idx(i):
                n_s = (i + 1) * 128
                work = idx_pool.tile([128, 1024], F32, name="work", tag="work")
                for h in range(HI):
                    m, r = h // 2, (h % 2) * 64
                    dst = work if h == 0 else idx_pool.tile([128, 1024], F32, name="aw", tag="aw")
                    for grp in range((n_s + 511) // 512):
                        ns0, ns1 = grp * 512, min(n_s, (grp + 1) * 512)
                        ps = mm_ps.tile([128, 512], F32, name="ps", tag="mm")
                        ki_rhs = kiw[0:64, ns0:ns1] if r == 0 else kiw2[64:128, ns0:ns1]
                        nc.tensor.matmul(
                            ps[:, 0:ns1 - ns0],
                            qiT[r:r + 64, m * 1024 + i * 128: m * 1024 + (i + 1) * 128],
                            ki_rhs,
                            start=True, stop=True)
                        nc.scalar.activation(dst[:, ns0:ns1], ps[:, 0:ns1 - ns0], AF.Relu,
                                             scale=w8[:, i * 4 + h: i * 4 + h + 1])
                    if h > 0:
                        nc.gpsimd.tensor_tensor(work[:, 0:n_s], work[:, 0:n_s], dst[:, 0:n_s], op=ALU.add)
                nc.gpsimd.tensor_tensor(work[:, i * 128:(i + 1) * 128],
                                        work[:, i * 128:(i + 1) * 128], causal[:], op=ALU.add)
                tmax = idx_pool.tile([128, 8], F32, name="tmax", tag="tmax")
                for _ in range(8):
                    nc.vector.max(tmax[:], work[:, 0:n_s])
                    nc.vector.match_replace(work[:, 0:n_s], tmax[:], work[:, 0:n_s], SENT)
                mk = mask_tiles[i]
                nc.vector.tensor_scalar(mk[:], work[:, 0:n_s], SENT, NEG,
                                        op0=ALU.not_equal, op1=ALU.mult)
                nc.gpsimd.tensor_tensor(mk[:, i * 128:(i + 1) * 128],
                                        mk[:, i * 128:(i + 1) * 128], causal[:], op=ALU.add)

            def emit_attn_tg(tg):
                i_lo, i_hi = tg * 4, tg * 4 + 4
                for h in range(HG):
                    m, r = h // 2, (h % 2) * 64
                    probTall = attn_pool.tile([128, NCHUNK * 512], MMDT, name="probTall", tag="probTall", bufs=2)
                    for i in range(i_lo, i_hi):
                        n_s = (i + 1) * 128
                        sc_sb = attn2_pool.tile([128, 1024], F32, name="sc_sb", tag="sc")
                        for grp in range((n_s + 511) // 512):
                            ns0, ns1 = grp * 512, min(n_s, (grp + 1) * 512)
                            ps = mm_ps.tile([128, 512], F32, name="ps", tag="mm")
                            nc.tensor.matmul(
                                ps[:, 0:ns1 - ns0],
                                qT[r:r + 64, m * 1024 + i * 128: m * 1024 + (i + 1) * 128],
                                kT[r:r + 64, m * 1024 + ns0: m * 1024 + ns1],
                                start=True, stop=True)
                            nc.vector.tensor_tensor(sc_sb[:, ns0:ns1], ps[:, 0:ns1 - ns0],
                                                    mask_tiles[i][:, ns0:ns1], op=ALU.add)
                        scr = attn2_pool.tile([128, 1024], F32, name="scr", tag="scr")
                        den = attn2_pool.tile([128, 1], F32, name="den", tag="den")
                        nc.scalar.activation(scr[:, 0:n_s], sc_sb[:, 0:n_s], AF.Exp,
                                             scale=0.125, accum_out=den[:])
                        rec = attn2_pool.tile([128, 1], F32, name="rec", tag="rec")
                        nc.vector.reciprocal(rec[:], den[:])
                        nc.gpsimd.tensor_scalar(sc_sb[:, 0:n_s], scr[:, 0:n_s], rec[:], None,
                                                op0=ALU.mult)
                        toff = (i - i_lo) * 128
                        for bi in range((i + 4) // 4):
                            cnt = min(i + 1, bi * 4 + 4) - bi * 4
                            pt = tp_ps.tile([128, 512], F32, name="pt", tag="tp")
                            for q in range(cnt):
                                sc = bi * 4 + q
                                nc.tensor.matmul(pt[:, q * 128:(q + 1) * 128],
                                                 sc_sb[:, sc * 128:(sc + 1) * 128], ident[:],
                                                 is_transpose=True,
                                                 start=(q == 0), stop=(q == cnt - 1))
                            if cnt > 1:
                                base = bi * 4 * 512
                                dst = probTall[:, base: base + cnt * 512]
                                dst = dst.rearrange("p (c q) -> p c q", q=512)[:, :, toff:toff + 128]
                                srcv = pt[:, 0:cnt * 128].rearrange("p (c q) -> p c q", q=128)
                            else:
                                dst = probTall[:, bi * 4 * 512 + toff: bi * 4 * 512 + toff + 128]
                                srcv = pt[:, 0:128]
                            nc.scalar.copy(dst, srcv)
                    pc = ctx_ps.tile([64, 512], F32, name="pc", tag="ctx")
                    n_sc = i_hi
                    for sc in range(n_sc):
                        off = max(sc - i_lo, 0) * 128
                        nc.tensor.matmul(
                            pc[:, off:512],
                            v_sb[:, sc * 256 + h * 64: sc * 256 + (h + 1) * 64],
                            probTall[:, sc * 512 + off: sc * 512 + 512],
                            start=(sc == 0), stop=(sc == n_sc - 1))
                    ck, rr = h // 2, (h % 2) * 64
                    nc.scalar.copy(ctxT[rr:rr + 64, ck * 1024 + tg * 512: ck * 1024 + (tg + 1) * 512], pc[:])
                for i in range(i_lo, i_hi):
                    out_sb = attn2_pool.tile([128, 1024], F32, name="out_sb", tag="out")
                    for og in range(2):
                        ps = mm_ps.tile([128, 512], F32, name="ps", tag="mm")
                        for ck in range(2):
                            nc.tensor.matmul(
                                ps[:],
                                ctxT[:, ck * 1024 + i * 128: ck * 1024 + (i + 1) * 128],
                                wo_sb[:, ck * 1024 + og * 512: ck * 1024 + (og + 1) * 512],
                                start=(ck == 0), stop=(ck == 1))
                        nc.scalar.copy(out_sb[:, og * 512:(og + 1) * 512], ps[:])
                    nc.sync.dma_start(out_ap[i * 128:(i + 1) * 128, :], out_sb[:])

            for i in range(NCHUNK):
                emit_idx(i)
            for tg in range(2):
                emit_attn_tg(tg)

    stack.close()


def _build_nc(loop=0):
    nc = bacc.Bacc("TRN2")
    x = nc.dram_tensor("x", [T, D], F32, kind="ExternalInput")
    wq = nc.dram_tensor("wq", [D, 256], F32, kind="ExternalInput")
    wk = nc.dram_tensor("wk", [D, 256], F32, kind="ExternalInput")
    wv = nc.dram_tensor("wv", [D, 256], F32, kind="ExternalInput")
    wo = nc.dram_tensor("wo", [256, D], F32, kind="ExternalInput")
    wi = nc.dram_tensor("wi", [D, 324], F32, kind="ExternalInput")
    out = nc.dram_tensor("out", [T, D], F32, kind="ExternalOutput")
    with tile.TileContext(nc) as tc:
        if loop:
            with tc.For_i(0, loop, 1):
                build_kernel(tc, out.ap(), x.ap(), wq.ap(), wk.ap(), wv.ap(), wo.ap(), wi.ap())
        else:
            build_kernel(tc, out.ap(), x.ap(), wq.ap(), wk.ap(), wv.ap(), wo.ap(), wi.ap())
    nc.compile()
    return nc


def kernel(x, Wq, Wk, Wv, Wo, bo, Wqi, Wki, Ww, _trace=False):
    _install_neff_cache()
    x, Wq, Wk, Wv, Wo, bo, Wqi, Wki, Ww = (
        np.ascontiguousarray(np.asarray(a, np.float32))
        for a in (x, Wq, Wk, Wv, Wo, bo, Wqi, Wki, Ww))
    wi = np.ascontiguousarray(np.concatenate([Wqi, Wki, Ww], axis=1))
    nc = _build_nc()
    in_maps = []
    for b in range(B):
        for g in range(4):
            c = slice(g * 256, (g + 1) * 256)
            in_maps.append({
                "x": np.ascontiguousarray(x[b]),
                "wq": np.ascontiguousarray(Wq[:, c]),
                "wk": np.ascontiguousarray(Wk[:, c]),
                "wv": np.ascontiguousarray(Wv[:, c]),
                "wo": np.ascontiguousarray(Wo[c, :]),
                "wi": wi,
            })
    res = run_bass_kernel_spmd(nc, in_maps, core_ids=list(range(8)), trace=_trace)
    outs = [r["out"] for r in res.results]
    full = np.stack([sum(outs[b * 4:(b + 1) * 4]) + bo for b in range(B)], axis=0)
    full = full.astype(np.float32)
    if _trace:
        return full, res
    return full


def _make_in_maps(x, Wq, Wk, Wv, Wo, Wqi, Wki, Ww):
    wi = np.ascontiguousarray(np.concatenate([Wqi, Wki, Ww], axis=1))
    in_maps = []
    for b in range(B):
        for g in range(4):
            c = slice(g * 256, (g + 1) * 256)
            in_maps.append({
                "x": np.ascontiguousarray(x[b]),
                "wq": np.ascontiguousarray(Wq[:, c]),
                "wk": np.ascontiguousarray(Wk[:, c]),
                "wv": np.ascontiguousarray(Wv[:, c]),
                "wo": np.ascontiguousarray(Wo[c, :]),
                "wi": wi,
            })
    return in_maps


def bench_exec_ns(inputs, iters=10, loop=256):
    """Per-iteration device time: the kernel body loops `loop` times inside one
    NEFF; dispatch-overhead floor (loop=1 variant) is subtracted via the slope
    between two loop counts. Returns ns per kernel iteration."""
    lo = max(1, loop // 8)
    t_hi = _bench_exec_wall(inputs, iters, loop)
    t_lo = _bench_exec_wall(inputs, iters, lo)
    return (t_hi - t_lo) / (loop - lo) * 1e9


def _bench_exec_wall(inputs, iters, loop):
    import time

    import jax
    from jax.experimental.shard_map import shard_map
    from jax.sharding import Mesh, NamedSharding, PartitionSpec

    import concourse.bass2jax as b2j

    _install_neff_cache()
    b2j.install_neuronx_cc_hook()
    nc = _build_nc(loop=loop)
    ins = {k: np.ascontiguousarray(np.asarray(v, np.float32)) for k, v in inputs.items()}
    in_maps = _make_in_maps(ins["x"], ins["Wq"], ins["Wk"], ins["Wv"], ins["Wo"],
                            ins["Wqi"], ins["Wki"], ins["Ww"])

    partition_name = nc.partition_id_tensor.name if nc.partition_id_tensor else None
    in_names, out_names, out_avals, zero_outs = [], [], [], []
    for alloc in nc.m.functions[0].allocations:
        if not isinstance(alloc, mybir.MemoryLocationSet):
            continue
        name = alloc.memorylocations[0].name
        if alloc.kind == "ExternalInput":
            if name != partition_name:
                in_names.append(name)
        elif alloc.kind == "ExternalOutput":
            shape = tuple(alloc.tensor_shape)
            dtype = mybir.dt.np(alloc.dtype)
            out_names.append(name)
            out_avals.append(jax.core.ShapedArray(shape, dtype))
            zero_outs.append(np.zeros(shape, dtype))
    n_params = len(in_names)
    all_in_names = list(in_names) + list(out_names)
    if partition_name is not None:
        all_in_names.append(partition_name)

    def _body(*args):
        operands = list(args)
        if partition_name is not None:
            operands.append(b2j.partition_id_tensor())
        outs = b2j._bass_exec_p.bind(
            *operands,
            out_avals=tuple(out_avals),
            in_names=tuple(all_in_names),
            out_names=tuple(out_names),
            lowering_input_output_aliases=(),
            sim_require_finite=True,
            sim_require_nnan=True,
            nc=nc,
        )
        return tuple(outs)

    n_cores = len(in_maps)
    devices = jax.devices()[:n_cores]
    mesh = Mesh(np.asarray(devices), ("core",))
    in_specs = (PartitionSpec("core"),) * (n_params + len(out_names))
    out_specs = (PartitionSpec("core"),) * len(out_names)
    fn = jax.jit(shard_map(_body, mesh=mesh, in_specs=in_specs,
                           out_specs=out_specs, check_rep=False))
    sharding = NamedSharding(mesh, PartitionSpec("core"))
    dev_args = [
        jax.device_put(
            np.concatenate([np.asarray(in_maps[c][nm]) for c in range(n_cores)], axis=0),
            sharding)
        for nm in in_names
    ] + [
        jax.device_put(np.concatenate([z] * n_cores, axis=0), sharding)
        for z in zero_outs
    ]
    r = fn(*dev_args)
    jax.block_until_ready(r)
    times = []
    for _ in range(iters):
        t0 = time.perf_counter()
        r = fn(*dev_args)
        jax.block_until_ready(r)
        times.append(time.perf_counter() - t0)
    return min(times)


if __name__ == "__main__":
    rng = np.random.default_rng(0)
    ins = {
        "x": rng.standard_normal((B, T, D)).astype(np.float32),
        "Wq": (rng.standard_normal((D, D)) * 0.02).astype(np.float32),
        "Wk": (rng.standard_normal((D, D)) * 0.02).astype(np.float32),
        "Wv": (rng.standard_normal((D, D)) * 0.02).astype(np.float32),
        "Wo": (rng.standard_normal((D, D)) * 0.02).astype(np.float32),
        "bo": np.zeros(D, np.float32),
        "Wqi": (rng.standard_normal((D, HI * IHD)) * 0.02).astype(np.float32),
        "Wki": (rng.standard_normal((D, IHD)) * 0.02).astype(np.float32),
        "Ww": (rng.standard_normal((D, HI)) * 0.02).astype(np.float32),
    }
    out = kernel(**ins)
    print("out", out.shape, out.dtype, float(np.abs(out).max()))


# revision 37
# speedup vs baseline: 1.1717x; 1.0136x over previous
"""Bass/Trainium2 kernel for MultiHeadAttentionWithDSA (sparse attention with
lightning-indexer top-64 key selection), sharded over 8 NeuronCores.

Sharding: core = b*4 + g  (b in {0,1} batch, g in {0..3} head-group of 4 heads).
Each core computes a partial output  ctx_g @ Wo[g*256:(g+1)*256, :]  for its
batch; the host sums the 4 partials per batch and adds the bias.

All matmuls run in true fp32 (4 cycles/row) because the top-64 selection must
match the fp32 reference's ordering exactly at the boundaries.
"""

import numpy as np

import concourse.bacc as bacc
import concourse.bass as bass
import concourse.mybir as mybir
import concourse.tile as tile
from concourse import masks
from concourse.bass_utils import run_bass_kernel_spmd

F32 = mybir.dt.float32
F32R = mybir.dt.float32r
USE_F32R = True
MMDT = F32R if USE_F32R else F32
AF = mybir.ActivationFunctionType
ALU = mybir.AluOpType

B, T, D = 2, 1024, 1024
H, HD = 16, 64          # total heads, head dim
HG = 4                  # heads per core
HI, IHD = 4, 64         # index heads, index head dim
TOPK = 64
NCHUNK = T // 128       # 8 token chunks of 128
NEG = -3.0e30           # causal-invalid marker (additive mask value)
SENT = -1.0e30          # match_replace sentinel (distinct from NEG)

_NEFF_CACHE = "/var/tmp/bass-neff-cache"


def _install_neff_cache():
    """walrus compile output cache keyed on BIR hash (compiles are minutes)."""
    import hashlib
    import os
    import shutil

    import concourse.bass2jax as b2j

    if getattr(b2j, "_dsa_neff_cache_installed", False):
        return
    orig = b2j.compile_bir_kernel

    def cached(bir_json, tmpdir, neff_name="file.neff"):
        try:
            h = hashlib.sha256(
                bir_json if isinstance(bir_json, bytes) else bir_json.encode()
            ).hexdigest()[:24]
            os.makedirs(_NEFF_CACHE, exist_ok=True)
            hit = os.path.join(_NEFF_CACHE, h + ".neff")
            if os.path.exists(hit):
                dst = os.path.join(tmpdir, neff_name)
                shutil.copyfile(hit, dst)
                return dst
            neff = orig(bir_json, tmpdir, neff_name)
            shutil.copyfile(neff, hit + ".tmp")
            os.replace(hit + ".tmp", hit)
            return neff
        except OSError:
            return orig(bir_json, tmpdir, neff_name)

    b2j.compile_bir_kernel = cached
    b2j._dsa_neff_cache_installed = True


def build_kernel(tc, out_ap, x_ap, wq_ap, wk_ap, wv_ap, wo_ap, wi_ap):
    """Emit the per-core kernel. All APs are DRAM tensors:
    x [1024,1024], wq/wk/wv [1024,256], wo [256,1024],
    wi [1024,324] = concat(Wqi[1024,256], Wki[1024,64], Ww[1024,4]).
    out [1024,1024] partial (pre-bias, pre-reduction over head groups).
    """
    nc = tc.nc
    from contextlib import ExitStack
    stack = ExitStack()

    const_pool = stack.enter_context(tc.tile_pool(name="const", bufs=1))
    ident = const_pool.tile([128, 128], F32)
    masks.make_identity(nc, ident[:])
    causal = const_pool.tile([128, 128], F32)
    masks.make_causal_mask(nc, causal[:], mask_val=NEG)

    w_pool = stack.enter_context(tc.tile_pool(name="weights", bufs=1))
    wq_sb = w_pool.tile([128, 8 * 256], MMDT)
    wk_sb = w_pool.tile([128, 8 * 256], MMDT)
    wv_sb = w_pool.tile([128, 8 * 256], MMDT)
    wo_sb = w_pool.tile([128, 2 * 1024], MMDT)
    with tc.tile_pool(name="wload", bufs=2) as wload:
        for (ap_, dst_) in ((wq_ap, wq_sb), (wk_ap, wk_sb), (wv_ap, wv_sb)):
            for j in range(8):
                wt = wload.tile([128, 256], F32, name="wt", tag="wt")
                nc.sync.dma_start(wt[:], ap_[j * 128:(j + 1) * 128, :])
                nc.scalar.copy(dst_[:, j * 256:(j + 1) * 256], wt[:])
        for ck in range(2):
            wt2 = wload.tile([128, 1024], F32, name="wt2", tag="wt2")
            nc.sync.dma_start(wt2[:], wo_ap[ck * 128:(ck + 1) * 128, :])
            nc.scalar.copy(wo_sb[:, ck * 1024:(ck + 1) * 1024], wt2[:])

    act_pool = stack.enter_context(tc.tile_pool(name="acts", bufs=1))
    qT = act_pool.tile([128, 2 * 1024], MMDT)    # heads (2m,2m+1) rows, tokens free
    kT = act_pool.tile([128, 2 * 1024], MMDT)
    qiT = act_pool.tile([128, 2 * 1024], F32)
    kiw = act_pool.tile([128, 1024], F32)        # rows 0-63 kiT, 64-67 wT logits
    kiw2 = act_pool.tile([128, 1024], F32)       # rows 64-127: copy of kiT (odd index heads)
    v_sb = act_pool.tile([128, 8 * 256], MMDT)    # [s-chunk sc] at cols sc*256, head cols inside
    w8 = act_pool.tile([128, 32], F32)           # softmax(x@Ww)/8, chunk i at cols 4i
    mask_tiles = [act_pool.tile([128, (i + 1) * 128], F32, name=f"mask{i}", tag=f"mask{i}") for i in range(NCHUNK)]

    # ---- Phase A: load x, build xT via PE transposes ----
    with tc.tile_pool(name="tp_ps", bufs=2, space="PSUM") as tp_ps, \
         tc.tile_pool(name="mm_ps", bufs=4, space="PSUM") as mm_ps:
      with tc.tile_pool(name="xscope", bufs=1) as xscope, \
           tc.tile_pool(name="xtok", bufs=2) as xtok_pool:
        xT = xscope.tile([128, 8 * 1024], F32)   # [d-chunk j] at cols j*1024, feature-major
        xTr = xscope.tile([128, 8 * 1024], MMDT)  # rounded shadow for fp32r matmuls
        wi_sb = xscope.tile([128, 8 * 324], F32)
        for j in range(8):
            nc.sync.dma_start(wi_sb[:, j * 324:(j + 1) * 324], wi_ap[j * 128:(j + 1) * 128, :])

        for i in range(NCHUNK):
            xt = xtok_pool.tile([128, 1024], F32, tag="xtok")
            nc.sync.dma_start(xt[:], x_ap[i * 128:(i + 1) * 128, :])
            for bj in range(2):
                pt = tp_ps.tile([128, 512], F32, name="pt", tag="tp")
                for q in range(4):
                    j = bj * 4 + q
                    nc.tensor.matmul(pt[:, q * 128:(q + 1) * 128],
                                     xt[:, j * 128:(j + 1) * 128], ident[:],
                                     is_transpose=True, start=(q == 0), stop=(q == 3))
                dst = xT[:, bj * 4096: (bj + 1) * 4096]
                dst = dst.rearrange("p (c q) -> p c q", q=1024)[:, :, i * 128:(i + 1) * 128]
                nc.scalar.copy(dst, pt[:].rearrange("p (c q) -> p c q", q=128))
        for j in range(8):
            nc.vector.tensor_copy(xTr[:, j * 1024:(j + 1) * 1024], xT[:, j * 1024:(j + 1) * 1024])

        # ---- Phase B: projections (contract d over 8 chunks) ----
        # qT/kT/qiT: out [128 (2 heads x 64), t512] ; lhsT = W[:, m*128:+128]
        for (wsb, dst) in ((wq_sb, qT), (wk_sb, kT)):
            for m in range(2):
                for tg in range(2):
                    ps = mm_ps.tile([128, 512], F32, tag="mm")
                    for j in range(8):
                        nc.tensor.matmul(
                            ps[:],
                            wsb[:, j * 256 + m * 128: j * 256 + (m + 1) * 128],
                            xTr[:, j * 1024 + tg * 512: j * 1024 + (tg + 1) * 512],
                            start=(j == 0), stop=(j == 7))
                    nc.scalar.copy(dst[:, m * 1024 + tg * 512: m * 1024 + (tg + 1) * 512], ps[:])
        for m in range(2):  # qiT
            for tg in range(2):
                ps = mm_ps.tile([128, 512], F32, tag="mm")
                for j in range(8):
                    nc.tensor.matmul(
                        ps[:],
                        wi_sb[:, j * 324 + m * 128: j * 324 + (m + 1) * 128],
                        xT[:, j * 1024 + tg * 512: j * 1024 + (tg + 1) * 512],
                        start=(j == 0), stop=(j == 7))
                nc.scalar.copy(qiT[:, m * 1024 + tg * 512: m * 1024 + (tg + 1) * 512], ps[:])
        for tg in range(2):  # kiT + wT logits (68 cols of wi)
            ps = mm_ps.tile([128, 512], F32, tag="mm")
            for j in range(8):
                nc.tensor.matmul(
                    ps[0:68, :],
                    wi_sb[:, j * 324 + 256: j * 324 + 324],
                    xT[:, j * 1024 + tg * 512: j * 1024 + (tg + 1) * 512],
                    start=(j == 0), stop=(j == 7))
            nc.scalar.copy(kiw[0:68, tg * 512:(tg + 1) * 512], ps[0:68, :])
        nc.sync.dma_start(kiw2[64:128, :], kiw[0:64, :])
        # v natural layout: out [s128, 256]
        for sc in range(NCHUNK):
            ps = mm_ps.tile([128, 512], F32, tag="mm")
            for j in range(8):
                nc.tensor.matmul(
                    ps[:, 0:256],
                    xTr[:, j * 1024 + sc * 128: j * 1024 + (sc + 1) * 128],
                    wv_sb[:, j * 256:(j + 1) * 256],
                    start=(j == 0), stop=(j == 7))
            nc.scalar.copy(v_sb[:, sc * 256:(sc + 1) * 256], ps[:, 0:256])

        # w softmax per chunk: transpose wT logits [4, t128] -> [t128, 4]
        for i in range(NCHUNK):
            pw = tp_ps.tile([128, 128], F32, tag="tp")
            nc.tensor.transpose(pw[:, 0:4], kiw[64:68, i * 128:(i + 1) * 128], ident[64:68, 64:68])
            wexp = act_pool.tile([128, 4], F32, tag="wexp", bufs=2)
            wden = act_pool.tile([128, 1], F32, tag="wden", bufs=2)
            nc.scalar.activation(wexp[:], pw[:, 0:4], AF.Exp, accum_out=wden[:])
            wrec = act_pool.tile([128, 1], F32, tag="wrec", bufs=2)
            nc.vector.reciprocal(wrec[:], wden[:])
            nc.vector.tensor_scalar(w8[:, i * 4:(i + 1) * 4], wexp[:], wrec[:], 0.125,
                                    op0=ALU.mult, op1=ALU.mult)

      # ---- Phases C+D, interleaved per t-group: topk(tg+1) overlaps attention(tg) ----
      with tc.tile_pool(name="idx", bufs=3) as idx_pool, \
           tc.tile_pool(name="attn", bufs=1) as attn_pool, \
           tc.tile_pool(name="attn2", bufs=3) as attn2_pool, \
           tc.tile_pool(name="ctx_ps", bufs=2, space="PSUM") as ctx_ps:
            ctxT = attn_pool.tile([128, 2 * 1024], MMDT)  # [ck] at cols ck*1024

            def emit_idx(i):
                n_s = (i + 1) * 128
                work = idx_pool.tile([128, 1024], F32, name="work", tag="work")
                for h in range(HI):
                    m, r = h // 2, (h % 2) * 64
                    dst = work if h == 0 else idx_pool.tile([128, 1024], F32, name="aw", tag="aw")
                    for grp in range((n_s + 511) // 512):
                        ns0, ns1 = grp * 512, min(n_s, (grp + 1) * 512)
                        ps = mm_ps.tile([128, 512], F32, name="ps", tag="mm")
                        ki_rhs = kiw[0:64, ns0:ns1] if r == 0 else kiw2[64:128, ns0:ns1]
                        nc.tensor.matmul(
                            ps[:, 0:ns1 - ns0],
                            qiT[r:r + 64, m * 1024 + i * 128: m * 1024 + (i + 1) * 128],
                            ki_rhs,
                            start=True, stop=True)
                        nc.scalar.activation(dst[:, ns0:ns1], ps[:, 0:ns1 - ns0], AF.Relu,
                                             scale=w8[:, i * 4 + h: i * 4 + h + 1])
                    if h > 0:
                        nc.gpsimd.tensor_tensor(work[:, 0:n_s], work[:, 0:n_s], dst[:, 0:n_s], op=ALU.add)
                nc.gpsimd.tensor_tensor(work[:, i * 128:(i + 1) * 128],
                                        work[:, i * 128:(i + 1) * 128], causal[:], op=ALU.add)
                tmax = idx_pool.tile([128, 8], F32, name="tmax", tag="tmax")
                for _ in range(8):
                    nc.vector.max(tmax[:], work[:, 0:n_s])
                    nc.vector.match_replace(work[:, 0:n_s], tmax[:], work[:, 0:n_s], SENT)
                mk = mask_tiles[i]
                nc.vector.tensor_scalar(mk[:], work[:, 0:n_s], SENT, NEG,
                                        op0=ALU.not_equal, op1=ALU.mult)
                nc.gpsimd.tensor_tensor(mk[:, i * 128:(i + 1) * 128],
                                        mk[:, i * 128:(i + 1) * 128], causal[:], op=ALU.add)

            def emit_attn_tg(tg):
                i_lo, i_hi = tg * 4, tg * 4 + 4
                for h in range(HG):
                    m, r = h // 2, (h % 2) * 64
                    probTall = attn_pool.tile([128, NCHUNK * 512], MMDT, name="probTall", tag="probTall", bufs=2)
                    for i in range(i_lo, i_hi):
                        n_s = (i + 1) * 128
                        sc_sb = attn2_pool.tile([128, 1024], F32, name="sc_sb", tag="sc")
                        for grp in range((n_s + 511) // 512):
                            ns0, ns1 = grp * 512, min(n_s, (grp + 1) * 512)
                            ps = mm_ps.tile([128, 512], F32, name="ps", tag="mm")
                            nc.tensor.matmul(
                                ps[:, 0:ns1 - ns0],
                                qT[r:r + 64, m * 1024 + i * 128: m * 1024 + (i + 1) * 128],
                                kT[r:r + 64, m * 1024 + ns0: m * 1024 + ns1],
                                start=True, stop=True)
                            nc.vector.tensor_tensor(sc_sb[:, ns0:ns1], ps[:, 0:ns1 - ns0],
                                                    mask_tiles[i][:, ns0:ns1], op=ALU.add)
                        scr = attn2_pool.tile([128, 1024], F32, name="scr", tag="scr")
                        den = attn2_pool.tile([128, 1], F32, name="den", tag="den")
                        nc.scalar.activation(scr[:, 0:n_s], sc_sb[:, 0:n_s], AF.Exp,
                                             scale=0.125, accum_out=den[:])
                        rec = attn2_pool.tile([128, 1], F32, name="rec", tag="rec")
                        nc.vector.reciprocal(rec[:], den[:])
                        nc.gpsimd.tensor_scalar(sc_sb[:, 0:n_s], scr[:, 0:n_s], rec[:], None,
                                                op0=ALU.mult)
                        toff = (i - i_lo) * 128
                        for bi in range((i + 4) // 4):
                            cnt = min(i + 1, bi * 4 + 4) - bi * 4
                            pt = tp_ps.tile([128, 512], F32, name="pt", tag="tp")
                            for q in range(cnt):
                                sc = bi * 4 + q
                                nc.tensor.matmul(pt[:, q * 128:(q + 1) * 128],
                                                 sc_sb[:, sc * 128:(sc + 1) * 128], ident[:],
                                                 is_transpose=True,
                                                 start=(q == 0), stop=(q == cnt - 1))
                            if cnt > 1:
                                base = bi * 4 * 512
                                dst = probTall[:, base: base + cnt * 512]
                                dst = dst.rearrange("p (c q) -> p c q", q=512)[:, :, toff:toff + 128]
                                srcv = pt[:, 0:cnt * 128].rearrange("p (c q) -> p c q", q=128)
                            else:
                                dst = probTall[:, bi * 4 * 512 + toff: bi * 4 * 512 + toff + 128]
                                srcv = pt[:, 0:128]
                            nc.scalar.copy(dst, srcv)
                    pc = ctx_ps.tile([64, 512], F32, name="pc", tag="ctx")
                    n_sc = i_hi
                    for sc in range(n_sc):
                        off = max(sc - i_lo, 0) * 128
                        nc.tensor.matmul(
                            pc[:, off:512],
                            v_sb[:, sc * 256 + h * 64: sc * 256 + (h + 1) * 64],
                            probTall[:, sc * 512 + off: sc * 512 + 512],
                            start=(sc == 0), stop=(sc == n_sc - 1))
                    ck, rr = h // 2, (h % 2) * 64
                    nc.scalar.copy(ctxT[rr:rr + 64, ck * 1024 + tg * 512: ck * 1024 + (tg + 1) * 512], pc[:])
                for i in range(i_lo, i_hi):
                    out_sb = attn2_pool.tile([128, 1024], F32, name="out_sb", tag="out")
                    for og in range(2):
                        ps = mm_ps.tile([128, 512], F32, name="ps", tag="mm")
                        for ck in range(2):
                            nc.tensor.matmul(
                                ps[:],
                                ctxT[:, ck * 1024 + i * 128: ck * 1024 + (i + 1) * 128],
                                wo_sb[:, ck * 1024 + og * 512: ck * 1024 + (og + 1) * 512],
                                start=(ck == 0), stop=(ck == 1))
                        nc.scalar.copy(out_sb[:, og * 512:(og + 1) * 512], ps[:])
                    nc.sync.dma_start(out_ap[i * 128:(i + 1) * 128, :], out_sb[:])

            for i in range(NCHUNK):
                emit_idx(i)
            for tg in range(2):
                emit_attn_tg(tg)

    stack.close()


def _build_nc(loop=0):
    nc = bacc.Bacc("TRN2")
    x = nc.dram_tensor("x", [T, D], F32, kind="ExternalInput")
    wq = nc.dram_tensor("wq", [D, 256], F32, kind="ExternalInput")
    wk = nc.dram_tensor("wk", [D, 256], F32, kind="ExternalInput")
    wv = nc.dram_tensor("wv", [D, 256], F32, kind="ExternalInput")
    wo = nc.dram_tensor("wo", [256, D], F32, kind="ExternalInput")
    wi = nc.dram_tensor("wi", [D, 324], F32, kind="ExternalInput")
    out = nc.dram_tensor("out", [T, D], F32, kind="ExternalOutput")
    with tile.TileContext(nc) as tc:
        if loop:
            with tc.For_i(0, loop, 1):
                build_kernel(tc, out.ap(), x.ap(), wq.ap(), wk.ap(), wv.ap(), wo.ap(), wi.ap())
        else:
            build_kernel(tc, out.ap(), x.ap(), wq.ap(), wk.ap(), wv.ap(), wo.ap(), wi.ap())
    nc.compile()
    return nc


def kernel(x, Wq, Wk, Wv, Wo, bo, Wqi, Wki, Ww, _trace=False):
    _install_neff_cache()
    x, Wq, Wk, Wv, Wo, bo, Wqi, Wki, Ww = (
        np.ascontiguousarray(np.asarray(a, np.float32))
        for a in (x, Wq, Wk, Wv, Wo, bo, Wqi, Wki, Ww))
    wi = np.ascontiguousarray(np.concatenate([Wqi, Wki, Ww], axis=1))
    nc = _build_nc()
    in_maps = []
    for b in range(B):
        for g in range(4):
            c = slice(g * 256, (g + 1) * 256)
            in_maps.append({
                "x": np.ascontiguousarray(x[b]),
                "wq": np.ascontiguousarray(Wq[:, c]),
                "wk": np.ascontiguousarray(Wk[:, c]),
                "wv": np.ascontiguousarray(Wv[:, c]),
                "wo": np.ascontiguousarray(Wo[c, :]),
                "wi": wi,
            })
    res = run_bass_kernel_spmd(nc, in_maps, core_ids=list(range(8)), trace=_trace)
    outs = [r["out"] for r in res.results]
    full = np.stack([sum(outs[b * 4:(b + 1) * 4]) + bo for b in range(B)], axis=0)
    full = full.astype(np.float32)
    if _trace:
        return full, res
    return full


def _make_in_maps(x, Wq, Wk, Wv, Wo, Wqi, Wki, Ww):
    wi = np.ascontiguousarray(np.concatenate([Wqi, Wki, Ww], axis=1))
    in_maps = []
    for b in range(B):
        for g in range(4):
            c = slice(g * 256, (g + 1) * 256)
            in_maps.append({
                "x": np.ascontiguousarray(x[b]),
                "wq": np.ascontiguousarray(Wq[:, c]),
                "wk": np.ascontiguousarray(Wk[:, c]),
                "wv": np.ascontiguousarray(Wv[:, c]),
                "wo": np.ascontiguousarray(Wo[c, :]),
                "wi": wi,
            })
    return in_maps


def bench_exec_ns(inputs, iters=10, loop=256):
    """Per-iteration device time: the kernel body loops `loop` times inside one
    NEFF; dispatch-overhead floor (loop=1 variant) is subtracted via the slope
    between two loop counts. Returns ns per kernel iteration."""
    lo = max(1, loop // 8)
    t_hi = _bench_exec_wall(inputs, iters, loop)
    t_lo = _bench_exec_wall(inputs, iters, lo)
    return (t_hi - t_lo) / (loop - lo) * 1e9


def _bench_exec_wall(inputs, iters, loop):
    import time

    import jax
    from jax.experimental.shard_map import shard_map
    from jax.sharding import Mesh, NamedSharding, PartitionSpec

    import concourse.bass2jax as b2j

    _install_neff_cache()
    b2j.install_neuronx_cc_hook()
    nc = _build_nc(loop=loop)
    ins = {k: np.ascontiguousarray(np.asarray(v, np.float32)) for k, v in inputs.items()}
    in_maps = _make_in_maps(ins["x"], ins["Wq"], ins["Wk"], ins["Wv"], ins["Wo"],
                            ins["Wqi"], ins["Wki"], ins["Ww"])

    partition_name = nc.partition_id_tensor.name if nc.partition_id_tensor else None
    in_names, out_names, out_avals, zero_outs = [], [], [], []
    for alloc in nc.m.functions[0].allocations:
        if not isinstance(alloc, mybir.MemoryLocationSet):
            continue
        name = alloc.memorylocations[0].name
        if alloc.kind == "ExternalInput":
            if name != partition_name:
                in_names.append(name)
        elif alloc.kind == "ExternalOutput":
            shape = tuple(alloc.tensor_shape)
            dtype = mybir.dt.np(alloc.dtype)
            out_names.append(name)
            out_avals.append(jax.core.ShapedArray(shape, dtype))
            zero_outs.append(np.zeros(shape, dtype))
    n_params = len(in_names)
    all_in_names = list(in_names) + list(out_names)
    if partition_name is not None:
        all_in_names.append(partition_name)

    def _body(*args):
        operands = list(args)
        if partition_name is not None:
            operands.append(b2j.partition_id_tensor())
        outs = b2j._bass_exec_p.bind(
            *operands,
            out_avals=tuple(out_avals),
            in_names=tuple(all_in_names),
            out_names=tuple(out_names),
            lowering_input_output_aliases=(),
            sim_require_finite=True,
            sim_require_nnan=True,
            nc=nc,
        )
        return tuple(outs)

    n_cores = len(in_maps)
    devices = jax.devices()[:n_cores]
    mesh = Mesh(np.asarray(devices), ("core",))
    in_specs = (PartitionSpec("core"),) * (n_params + len(out_names))
    out_specs = (PartitionSpec("core"),) * len(out_names)
    fn = jax.jit(shard_map(_body, mesh=mesh, in_specs=in_specs,
                           out_specs=out_specs, check_rep=False))
    sharding = NamedSharding(mesh, PartitionSpec("core"))
    dev_args = [
        jax.device_put(
            np.concatenate([np.asarray(in_maps[c][nm]) for c in range(n_cores)], axis=0),
            sharding)
        for nm in in_names
    ] + [
        jax.device_put(np.concatenate([z] * n_cores, axis=0), sharding)
        for z in zero_outs
    ]
    r = fn(*dev_args)
    jax.block_until_ready(r)
    times = []
    for _ in range(iters):
        t0 = time.perf_counter()
        r = fn(*dev_args)
        jax.block_until_ready(r)
        times.append(time.perf_counter() - t0)
    return min(times)


if __name__ == "__main__":
    rng = np.random.default_rng(0)
    ins = {
        "x": rng.standard_normal((B, T, D)).astype(np.float32),
        "Wq": (rng.standard_normal((D, D)) * 0.02).astype(np.float32),
        "Wk": (rng.standard_normal((D, D)) * 0.02).astype(np.float32),
        "Wv": (rng.standard_normal((D, D)) * 0.02).astype(np.float32),
        "Wo": (rng.standard_normal((D, D)) * 0.02).astype(np.float32),
        "bo": np.zeros(D, np.float32),
        "Wqi": (rng.standard_normal((D, HI * IHD)) * 0.02).astype(np.float32),
        "Wki": (rng.standard_normal((D, IHD)) * 0.02).astype(np.float32),
        "Ww": (rng.standard_normal((D, HI)) * 0.02).astype(np.float32),
    }
    out = kernel(**ins)
    print("out", out.shape, out.dtype, float(np.abs(out).max()))
